# revision 1
# baseline (speedup 1.0000x reference)
"""Causal self-attention (B=2, T=2048, C=1024, H=16, RoPE) on 8 TRN2 NeuronCores.

Sharding: core i handles batch b = i//4 and head group g = i%4 (4 heads each).
Each core computes q/k (transposed, RoPE'd), v, causal attention, and a partial
output projection; the host sums the 4 partials per batch element (tensor-
parallel unshard) and adds the constant term b_proj + b_v @ W_proj, which is
independent of x because softmax rows sum to 1.

Layout strategy (no on-chip transposes):
  - host passes x^T  [C, T]
  - q^T, k^T computed as (W^T x^T) with j (head*dim) on partitions
  - rotate_half(q) computed on-chip as P @ q^T (signed permutation matmul)
  - v computed in natural [t, j] layout, augmented with a ones column so the
    attention-value matmul also produces the softmax denominator
  - scores computed transposed: s^T[k, q] = k^T(d,k)^T . q^T(d,q); softmax
    normalization deferred until after AV (flash-style), no max subtraction
    (scores are ~N(0,1); exp is safe in fp32)
  - output projection consumes y^T directly: out[t, c] = y^T(j,t)^T . Wp(j,c)
Matmul operands are bf16 (full PE rate incl. 512-wide moving operands --
measured f32r falls to half rate there); softmax/RoPE arithmetic stays f32.
"""

import numpy as np

B, T, C, H, D = 2, 2048, 1024, 16, 64
G = 4           # heads per core
NCORES = 8
TS = 512        # t / q super-tile width
NT = T // 128   # 16 t-blocks
NTS = T // TS   # 4 t-supers
MASK_VAL = -1e5

_cached = {}


def _apply_workarounds():
    """This neuronxcc build rejects TPB instructions with >1 embedded sem wait.
    Patch the Tile drain and add a BIR pass splitting extra waits into
    standalone EventSemaphore instructions on the same (in-order) engine."""
    import concourse.tile as tile
    import concourse.mybir as mybir
    from concourse.vector_clock import ScopedClock

    if getattr(tile.TileContext, "_multiwait_patched", False):
        return

    def _drain_and_barrier(self, tick_clock, wait_clock):
        nc = self.nc
        probe = nc.sync.nop(nofuse=True)
        wait_clock.add_sem_waits(probe.ins, ScopedClock({None: tick_clock.global_clock}))
        si = probe.ins.sync_info
        waits = list(si.on_wait) if si and si.on_wait else []
        if si is not None:
            si.on_wait = []
        by_num = {h.num: h for h in self.sems.allocated().values()}
        for w in waits:
            nc.sync.wait_ge(by_num[w.id], w.wait_value)
        nc.sync.drain()
        nc.all_engine_barrier()
        popped = nc._tile_sem_poison_stack.pop()
        assert popped is self._sem_poison
        nc.clear_and_free_semaphores(list(self.sems.allocated().values()))
        nc.all_engine_barrier()

    tile.TileContext._drain_and_barrier = _drain_and_barrier
    tile.TileContext._multiwait_patched = True


def _split_multiwaits(nc, maxw=1):
    import concourse.mybir as mybir

    n = 0
    for f in nc.m.functions:
        for bb in f.blocks:
            insts = list(bb.instructions)
            out = []
            changed = False
            for inst in insts:
                si = inst.sync_info
                waits = list(si.on_wait) if si and si.on_wait else []
                if len(waits) > maxw:
                    for k, w in enumerate(waits[: len(waits) - maxw]):
                        out.append(
                            mybir.InstEventSemaphore(
                                name=f"{inst.name}-xw{k}",
                                engine=inst.engine,
                                ins=[],
                                outs=[],
                                sync_info=mybir.SyncInfo(on_wait=[w], on_update=[]),
                            )
                        )
                        n += 1
                    si.on_wait = waits[len(waits) - maxw :]
                    changed = True
                out.append(inst)
            if changed:
                bb.instructions.clear()
                for i in out:
                    bb.add_instruction(i)
    return n


def _build():
    import concourse.bass as bass
    import concourse.mybir as mybir
    import concourse.tile as tile

    _apply_workarounds()

    f32 = mybir.dt.float32
    bf16 = mybir.dt.bfloat16
    Exp = mybir.ActivationFunctionType.Exp
    Ident = mybir.ActivationFunctionType.Identity


    nc = bass.Bass()

    xT = nc.dram_tensor("xT", [C, T], bf16, kind="ExternalInput")
    w1 = nc.dram_tensor("w1", [C, 512], bf16, kind="ExternalInput")     # [q01 q23 k01 k23]
    b1 = nc.dram_tensor("b1", [128, 4], f32, kind="ExternalInput")
    wv = nc.dram_tensor("wv", [C, 256], bf16, kind="ExternalInput")
    wp = nc.dram_tensor("wp", [256, C], bf16, kind="ExternalInput")
    cosb = nc.dram_tensor("cosb", [128, T], f32, kind="ExternalInput")
    sinb = nc.dram_tensor("sinb", [128, T], f32, kind="ExternalInput")
    masks = nc.dram_tensor("masks", [128, 4 * TS], bf16, kind="ExternalInput")
    pt2 = nc.dram_tensor("pt2", [128, 128], bf16, kind="ExternalInput")  # rotate-half perm^T
    out = nc.dram_tensor("out", [T, C], f32, kind="ExternalOutput")
    scr = nc.dram_tensor("scr", [16, TS], f32)                          # S bounce
    scr2 = nc.dram_tensor("scr2", [16, TS], f32)                        # 1/S bounce

    with tile.TileContext(nc) as tc:
        with (
            tc.tile_pool(name="persist", bufs=1) as per,
            tc.tile_pool(name="xq", bufs=12) as xq,
            tc.tile_pool(name="qkr", bufs=4) as qkrp,
            tc.tile_pool(name="tmp", bufs=4) as tmp,
            tc.tile_pool(name="pp", bufs=6) as pp,
            tc.tile_pool(name="rr", bufs=4) as rr,
            tc.tile_pool(name="yo", bufs=4) as yo,
            tc.tile_pool(name="psA", bufs=4, space="PSUM") as psA,
            tc.tile_pool(name="psO", bufs=4, space="PSUM") as psO,
        ):
            # ---- persistent tiles ----
            w1_s = per.tile([128, 8, 512], bf16)
            wv_s = per.tile([128, 8, 256], bf16)
            wp_s = per.tile([128, 2, C], bf16)
            b1_s = per.tile([128, 4], f32)
            cos_s = per.tile([128, T], f32)
            sin_s = per.tile([128, T], f32)
            msk_s = per.tile([128, 4 * TS], bf16)
            pt2_s = per.tile([128, 128], bf16)
            qk_s = per.tile([128, 4, T], bf16)        # [q01' q23' k01' k23']
            # v storage per head pair:
            #   [v_even(0:64) | ones(64:66) | gap(66:97) | v_odd(97:161)]
            # The AV lhsT is a 128-wide window: even head -> cols 0..127, so y
            # lands in psum rows 0..63 with the denominator in row 64; odd
            # head -> cols 33..160, so y lands in rows 64..127 with the
            # denominator (ones col 65) in row 32. Engine ops require
            # 32-aligned partition starts, so denominator rows must be 32/64.
            v_s = per.tile([128, NT, 2, 161], bf16)
            yT_s = per.tile([128, 2, T], bf16)

            for cb in range(8):
                nc.sync.dma_start(out=w1_s[:, cb, :], in_=w1[cb * 128:(cb + 1) * 128, :])
                nc.sync.dma_start(out=wv_s[:, cb, :], in_=wv[cb * 128:(cb + 1) * 128, :])
            for jb in range(2):
                nc.sync.dma_start(out=wp_s[:, jb, :], in_=wp[jb * 128:(jb + 1) * 128, :])
            nc.sync.dma_start(out=b1_s, in_=b1[:])
            nc.sync.dma_start(out=cos_s, in_=cosb[:])
            nc.sync.dma_start(out=sin_s, in_=sinb[:])
            nc.sync.dma_start(out=msk_s, in_=masks[:])
            nc.sync.dma_start(out=pt2_s, in_=pt2[:])
            nc.vector.memset(v_s.rearrange("p a b c -> p (a b c)"), 1.0)

            # ---- attention for one q-super (called as soon as its
            # projections exist, so PE fills ACT-bound stretches with the
            # next t-super's projection matmuls) ----
            def do_attention(js):
                qsl = slice(js * TS, (js + 1) * TS)
                nkb = 4 * js + 4
                for h in range(G):
                    par = h % 2
                    prow = slice(par * 64, par * 64 + 64)
                    srow = 64 - 32 * par  # denominator row (32-aligned)
                    qT = qk_s[prow, h // 2, :]
                    kT = qk_s[prow, 2 + h // 2, :]

                    def v_win(kb, pair=h // 2, par=par):
                        # 128-wide lhsT window into the [v_even |1|1| v_odd] slot
                        return v_s[:, kb, pair, 33 * par:33 * par + 128]

                    po = psO.tile([128, 512], f32, tag="av")
                    pend = None
                    for kb in range(nkb):
                        pss = psA.tile([128, 512], f32, tag="mm")
                        nc.tensor.matmul(
                            pss,
                            kT[:, kb * 128:(kb + 1) * 128],
                            qT[:, qsl],
                            start=True,
                            stop=True,
                        )
                        pt = pp.tile([128, TS], bf16, tag="pt")
                        roff = kb - 4 * js
                        if roff >= 0:
                            # columns qq < 128*roff are fully masked: skip exp
                            # there and zero them; the partially-masked rest is
                            # zeroed multiplicatively after exp (bf16 SBUF TT
                            # is cheaper than an f32 PSUM-operand mask add).
                            w0 = 128 * roff
                            if w0:
                                nc.vector.memset(pt[:, :w0], 0.0)
                            nc.scalar.activation(pt[:, w0:], pss[:, w0:], Exp, scale=0.125)
                            nc.vector.tensor_mul(
                                pt[:, w0:], pt[:, w0:],
                                msk_s[:, roff * TS + w0:(roff + 1) * TS],
                            )
                        else:
                            nc.scalar.activation(pt, pss, Exp, scale=0.125)
                        if pend is not None:
                            nc.tensor.matmul(po, v_win(kb - 1), pend,
                                             start=(kb == 1), stop=False)
                        pend = pt
                    nc.tensor.matmul(po, v_win(nkb - 1), pend,
                                     start=False, stop=True)
                    # normalize y = po * (1/S). DVE InstReciprocal costs ~6.3
                    # cyc/elem along the free dim, so spread S across 64
                    # partitions (DRAM bounce) before taking the reciprocal,
                    # then bounce back as a partition-broadcast row.
                    slot = h * 4 + js
                    rs = rr.tile([65, TS], f32, tag="rs")
                    nc.vector.tensor_copy(rs[srow:srow + 1, :], po[srow:srow + 1, :])
                    nc.sync.dma_start(out=scr[slot:slot + 1, :], in_=rs[srow:srow + 1, :])
                    rv = rr.tile([64, 8], f32, tag="rv")
                    nc.sync.dma_start(
                        out=rv,
                        in_=scr[slot:slot + 1, :].rearrange("a (p f) -> (a p) f", p=64),
                    )
                    rvr = rr.tile([64, 8], f32, tag="rvr")
                    nc.vector.reciprocal(rvr, rv)
                    nc.sync.dma_start(
                        out=scr2[slot:slot + 1, :].rearrange("a (p f) -> (a p) f", p=64),
                        in_=rvr,
                    )
                    rb = rr.tile([128, TS], f32, tag="rb")
                    sc = scr2[slot:slot + 1, :]
                    nc.gpsimd.dma_start(
                        out=rb[prow, :],
                        in_=bass.AP(tensor=sc.tensor, offset=sc.offset,
                                    ap=[[0, 64]] + list(sc.ap[1:])),
                    )
                    nc.vector.tensor_mul(yT_s[prow, h // 2, qsl], po[prow, :], rb[prow, :])

                # output projection deferred one q-super so the PE never
                # waits on the normalize chain (copy->DMA->recip->DMA->DMA->TT)
                oproj_js = js - 1 if js >= 1 else None
                if js == NTS - 1:
                    oproj_tbs = list(range(4 * (js - 1), 4 * js)) + list(range(4 * js, 4 * js + 4))
                elif js >= 1:
                    oproj_tbs = list(range(4 * (js - 1), 4 * js))
                else:
                    oproj_tbs = []
                for tb in oproj_tbs:
                    for cs in range(2):
                        py = psA.tile([128, 512], f32, tag="mm")
                        for jb in range(2):
                            nc.tensor.matmul(
                                py,
                                yT_s[:, jb, tb * 128:(tb + 1) * 128],
                                wp_s[:, jb, cs * 512:(cs + 1) * 512],
                                start=(jb == 0),
                                stop=(jb == 1),
                            )
                        ot = yo.tile([128, 512], f32, tag="ot")
                        nc.vector.tensor_copy(ot, py)
                        nc.sync.dma_start(
                            out=out[tb * 128:(tb + 1) * 128, cs * 512:(cs + 1) * 512],
                            in_=ot,
                        )

            # ---- phase 1: q/k projection + RoPE, v projection ----
            for ts in range(NTS):
                tsl = slice(ts * TS, (ts + 1) * TS)
                xts = []
                for cb in range(8):
                    xt = xq.tile([128, TS], bf16, tag="xts")
                    nc.sync.dma_start(out=xt, in_=xT[cb * 128:(cb + 1) * 128, tsl])
                    xts.append(xt)
                for jb in range(4):
                    ps = psA.tile([128, 512], f32, tag="mm")
                    for cb in range(8):
                        nc.tensor.matmul(
                            ps,
                            w1_s[:, cb, jb * 128:(jb + 1) * 128],
                            xts[cb],
                            start=(cb == 0),
                            stop=(cb == 7),
                        )
                    qkr = qkrp.tile([128, TS], bf16, tag="qkr")
                    nc.scalar.activation(qkr, ps, Ident, bias=b1_s[:, jb:jb + 1], scale=1.0)
                    psr = psA.tile([128, 512], f32, tag="mm")
                    nc.tensor.matmul(psr, pt2_s, qkr, start=True, stop=True)
                    t1 = tmp.tile([128, TS], f32, tag="t1")
                    nc.vector.tensor_mul(t1, qkr, cos_s[:, tsl])
                    t2 = tmp.tile([128, TS], f32, tag="t2")
                    nc.vector.tensor_mul(t2, psr, sin_s[:, tsl])
                    nc.vector.tensor_add(qk_s[:, jb, tsl], t1, t2)
                for tb2 in range(4):
                    tb = ts * 4 + tb2
                    psv = psA.tile([128, 512], f32, tag="mm")
                    for cb in range(8):
                        nc.tensor.matmul(
                            psv[:, :256],
                            xts[cb][:, tb2 * 128:(tb2 + 1) * 128],
                            wv_s[:, cb, :],
                            start=(cb == 0),
                            stop=(cb == 7),
                        )
                    psv4 = psv[:, :256].rearrange("p (pr par d) -> p pr par d", par=2, d=D)
                    nc.vector.tensor_copy(v_s[:, tb, :, 0:64], psv4[:, :, 0, :])
                    nc.vector.tensor_copy(v_s[:, tb, :, 97:161], psv4[:, :, 1, :])

                do_attention(ts)

    _split_multiwaits(nc)
    return nc


def _rot_cols(w):
    """rotate_half as a column transform: out[:, d] = -w[:, d+32] (d<32), w[:, d-32] (d>=32)."""
    o = np.empty_like(w)
    o[..., :32] = -w[..., 32:64]
    o[..., 32:] = w[..., :32]
    return o


def _host_inputs(x, W_attn, b_attn, W_proj):
    f32 = np.float32
    inv = (1.0 / (10000.0 ** (np.arange(0, D, 2, dtype=f32) / f32(D)))).astype(f32)
    t = np.arange(T, dtype=f32)
    ang = np.outer(inv, t).astype(f32)            # [32, T]
    cos32, sin32 = np.cos(ang).astype(f32), np.sin(ang).astype(f32)
    cosb = np.tile(cos32, (4, 1))                  # [128, T], row p -> freq p%32
    sinb = np.tile(sin32, (4, 1))

    kk = np.arange(128)[:, None]
    qq = np.arange(TS)[None, :]
    masks = np.concatenate(
        [np.where(qq >= kk + 128 * rr_, f32(1), f32(0)) for rr_ in range(4)],
        axis=1,
    )                                              # [128, 4*TS] multiplicative

    import ml_dtypes

    bf16 = ml_dtypes.bfloat16
    p64 = np.zeros((D, D), dtype=f32)
    for d in range(32):
        p64[d, d + 32] = -1.0
        p64[d + 32, d] = 1.0
    pt2 = np.zeros((128, 128), dtype=f32)
    pt2[:64, :64] = p64.T
    pt2[64:, 64:] = p64.T
    pt2 = pt2.astype(bf16)

    xTs = [np.ascontiguousarray(x[b].T).astype(bf16) for b in range(B)]

    per_g = []
    for g in range(G):
        hs = [4 * g + j for j in range(G)]
        qcols = [W_attn[:, h * D:(h + 1) * D] for h in hs]
        kcols = [W_attn[:, C + h * D:C + (h + 1) * D] for h in hs]
        qb = [b_attn[h * D:(h + 1) * D] for h in hs]
        kb_ = [b_attn[C + h * D:C + (h + 1) * D] for h in hs]
        w1 = np.concatenate(
            [qcols[0], qcols[1], qcols[2], qcols[3], kcols[0], kcols[1], kcols[2], kcols[3]],
            axis=1,
        ).astype(bf16)                             # [C, 512]: [q01 q23 k01 k23]
        b1 = np.concatenate(qb + kb_).astype(f32).reshape(4, 128).T.copy()  # [128, 4]
        wv_ = W_attn[:, 2 * C + 256 * g:2 * C + 256 * (g + 1)].astype(bf16)
        wp_ = W_proj[256 * g:256 * (g + 1), :].astype(bf16)
        per_g.append((w1, b1, wv_, wp_))

    shared = dict(cosb=cosb, sinb=sinb, masks=masks.astype(bf16), pt2=pt2)
    in_maps = []
    for i in range(NCORES):
        b, g = i // 4, i % 4
        w1, b1, wv_, wp_ = per_g[g]
        in_maps.append(dict(xT=xTs[b], w1=w1, b1=b1, wv=wv_, wp=wp_, **shared))
    return in_maps


def kernel(x, W_attn, b_attn, W_proj, b_proj):
    from concourse.bass_utils import run_bass_kernel_spmd

    x = np.asarray(x, dtype=np.float32)
    W_attn = np.asarray(W_attn, dtype=np.float32)
    b_attn = np.asarray(b_attn, dtype=np.float32)
    W_proj = np.asarray(W_proj, dtype=np.float32)
    b_proj = np.asarray(b_proj, dtype=np.float32)

    if "nc" not in _cached:
        _cached["nc"] = _build()
    nc = _cached["nc"]

    in_maps = _host_inputs(x, W_attn, b_attn, W_proj)
    res = run_bass_kernel_spmd(nc, in_maps, core_ids=list(range(NCORES)))
    _cached["last_results"] = res

    const = (b_proj + b_attn[2 * C:] @ W_proj).astype(np.float32)
    y = np.empty((B, T, C), dtype=np.float32)
    for b in range(B):
        acc = res.results[4 * b]["out"].astype(np.float32).copy()
        for g in range(1, 4):
            acc += res.results[4 * b + g]["out"]
        y[b] = acc + const
    return y



# revision 8
# speedup vs baseline: 1.1096x; 1.1096x over previous
"""Causal self-attention (B=2, T=2048, C=1024, H=16, RoPE) on 8 TRN2 NeuronCores.

Sharding: core i handles batch b = i//4 and head group g = i%4 (4 heads each).
Each core computes q/k (transposed, RoPE'd), v, causal attention, and a partial
output projection; the host sums the 4 partials per batch element (tensor-
parallel unshard) and adds the constant term b_proj + b_v @ W_proj, which is
independent of x because softmax rows sum to 1.

Layout strategy (no on-chip transposes):
  - host passes x^T  [C, T]
  - q^T, k^T computed as (W^T x^T) with j (head*dim) on partitions
  - rotate_half(q) computed on-chip as P @ q^T (signed permutation matmul)
  - v computed in natural [t, j] layout into per-t-block slots of a single
    v_s strip; the AV stationary operand is a two-block access pattern
    [v_head(64) | ones/pad(64)] so the same matmul also produces the softmax
    denominator (flash-style deferred normalization, no max subtraction)
  - scores computed transposed: s^T[k, q] = k^T(d,k)^T . q^T(d,q)
  - causal structure exploited at 128-column granularity: matmul/exp/AV are
    column-windowed on the diagonal q-super so fully-masked regions are never
    computed; the partial 128x128 diagonal block is masked multiplicatively
  - attention is software-pipelined at head granularity (QK chain of head h
    runs on PE while exps of head h-1 drain into its AV chain) and the output
    projection of q-super js-1 is interleaved between heads of q-super js
Matmul operands are bf16; softmax stays f32; RoPE tables and adds in bf16.
"""

import numpy as np

B, T, C, H, D = 2, 2048, 1024, 16, 64
G = 4           # heads per core
NCORES = 8
TS = 512        # t / q super-tile width
NT = T // 128   # 16 t-blocks
NTS = T // TS   # 4 t-supers

_cached = {}


def _apply_workarounds():
    """This neuronxcc build rejects TPB instructions with >1 embedded sem wait.
    Patch the Tile drain and add a BIR pass splitting extra waits into
    standalone EventSemaphore instructions on the same (in-order) engine."""
    import concourse.tile as tile
    import concourse.mybir as mybir
    from concourse.vector_clock import ScopedClock

    if getattr(tile.TileContext, "_multiwait_patched", False):
        return

    def _drain_and_barrier(self, tick_clock, wait_clock):
        nc = self.nc
        probe = nc.sync.nop(nofuse=True)
        wait_clock.add_sem_waits(probe.ins, ScopedClock({None: tick_clock.global_clock}))
        si = probe.ins.sync_info
        waits = list(si.on_wait) if si and si.on_wait else []
        if si is not None:
            si.on_wait = []
        by_num = {h.num: h for h in self.sems.allocated().values()}
        for w in waits:
            nc.sync.wait_ge(by_num[w.id], w.wait_value)
        nc.sync.drain()
        nc.all_engine_barrier()
        popped = nc._tile_sem_poison_stack.pop()
        assert popped is self._sem_poison
        nc.clear_and_free_semaphores(list(self.sems.allocated().values()))
        nc.all_engine_barrier()

    tile.TileContext._drain_and_barrier = _drain_and_barrier
    tile.TileContext._multiwait_patched = True


def _split_multiwaits(nc, maxw=1):
    import concourse.mybir as mybir

    n = 0
    for f in nc.m.functions:
        for bb in f.blocks:
            insts = list(bb.instructions)
            out = []
            changed = False
            for inst in insts:
                si = inst.sync_info
                waits = list(si.on_wait) if si and si.on_wait else []
                if len(waits) > maxw:
                    for k, w in enumerate(waits[: len(waits) - maxw]):
                        out.append(
                            mybir.InstEventSemaphore(
                                name=f"{inst.name}-xw{k}",
                                engine=inst.engine,
                                ins=[],
                                outs=[],
                                sync_info=mybir.SyncInfo(on_wait=[w], on_update=[]),
                            )
                        )
                        n += 1
                    si.on_wait = waits[len(waits) - maxw :]
                    changed = True
                out.append(inst)
            if changed:
                bb.instructions.clear()
                for i in out:
                    bb.add_instruction(i)
    return n


def _build():
    import concourse.bass as bass
    import concourse.mybir as mybir
    import concourse.tile as tile

    _apply_workarounds()

    f32 = mybir.dt.float32
    bf16 = mybir.dt.bfloat16
    Exp = mybir.ActivationFunctionType.Exp
    Ident = mybir.ActivationFunctionType.Identity

    nc = bass.Bass()

    xT = nc.dram_tensor("xT", [C, T], bf16, kind="ExternalInput")
    w1 = nc.dram_tensor("w1", [C, 512], bf16, kind="ExternalInput")     # [q01 q23 k01 k23]
    b1 = nc.dram_tensor("b1", [128, 4], f32, kind="ExternalInput")
    wv = nc.dram_tensor("wv", [C, 256], bf16, kind="ExternalInput")
    wp = nc.dram_tensor("wp", [256, C], bf16, kind="ExternalInput")
    cosb = nc.dram_tensor("cosb", [128, T], bf16, kind="ExternalInput")
    sinb = nc.dram_tensor("sinb", [128, T], bf16, kind="ExternalInput")
    msk = nc.dram_tensor("msk", [128, 128], bf16, kind="ExternalInput")
    pt2 = nc.dram_tensor("pt2", [128, 128], bf16, kind="ExternalInput")  # rotate-half perm^T
    out = nc.dram_tensor("out", [T, C], bf16, kind="ExternalOutput")
    scr = nc.dram_tensor("scr", [16, TS], f32)                          # S bounce
    scr2 = nc.dram_tensor("scr2", [16, TS], f32)                        # 1/S bounce

    with tile.TileContext(nc) as tc:
        with (
            tc.tile_pool(name="persist", bufs=1) as per,
            tc.tile_pool(name="xq", bufs=16) as xq,
            tc.tile_pool(name="qkr", bufs=3) as qkrp,
            tc.tile_pool(name="t1", bufs=2) as tp1,
            tc.tile_pool(name="t2", bufs=2) as tp2,
            tc.tile_pool(name="pt", bufs=34) as ptp,
            tc.tile_pool(name="rr", bufs=2) as rr,
            tc.tile_pool(name="rr2", bufs=3) as rr2,
            tc.tile_pool(name="rrb", bufs=2) as rrb,
            tc.tile_pool(name="yo", bufs=4) as yo,
            tc.tile_pool(name="psA", bufs=4, space="PSUM") as psA,
            tc.tile_pool(name="psO", bufs=2, space="PSUM") as psO,
            tc.tile_pool(name="psP", bufs=2, space="PSUM") as psP,
        ):
            # ---- persistent tiles ----
            w1_s = per.tile([128, 8, 512], bf16)
            wv_s = per.tile([128, 8, 256], bf16)
            wp_s = per.tile([128, 2, C], bf16)
            b1_s = per.tile([128, 4], f32)
            cos_s = per.tile([128, T], bf16)
            sin_s = per.tile([128, T], bf16)
            msk_s = per.tile([128, 128], bf16)
            pt2_s = per.tile([128, 128], bf16)
            qk_s = per.tile([128, 4, T], bf16)        # [q01' q23' k01' k23']
            # v storage per head pair:
            #   [v_even(0:64) | ones(64:66) | gap(66:97) | v_odd(97:161)]
            # The AV lhsT is a 128-wide window: even head -> cols 0..127, so y
            # lands in psum rows 0..63 with the denominator in row 64; odd
            # head -> cols 33..160, so y lands in rows 64..127 with the
            # denominator (ones col 65) in row 32. Engine ops require
            # 32-aligned partition starts, so denominator rows must be 32/64.
            v_s = per.tile([128, NT, 2, 161], bf16)
            yT_s = per.tile([128, 2, T], bf16)

            # ---- startup DMAs in priority order: the first projection chain
            # needs w1 block cb + x block cb, so interleave them ----
            xts0 = []
            for cb in range(8):
                nc.sync.dma_start(out=w1_s[:, cb, :], in_=w1[cb * 128:(cb + 1) * 128, :])
                xt = xq.tile([128, TS], bf16, tag="xts")
                nc.sync.dma_start(out=xt, in_=xT[cb * 128:(cb + 1) * 128, 0:TS])
                xts0.append(xt)
            nc.sync.dma_start(out=b1_s, in_=b1[:])
            nc.sync.dma_start(out=pt2_s, in_=pt2[:])
            nc.sync.dma_start(out=cos_s, in_=cosb[:])
            nc.sync.dma_start(out=sin_s, in_=sinb[:])
            nc.sync.dma_start(out=msk_s, in_=msk[:])
            for cb in range(8):
                nc.sync.dma_start(out=wv_s[:, cb, :], in_=wv[cb * 128:(cb + 1) * 128, :])
            for jb in range(2):
                nc.sync.dma_start(out=wp_s[:, jb, :], in_=wp[jb * 128:(jb + 1) * 128, :])
            nc.gpsimd.memset(v_s.rearrange("p a b c -> p (a b c)"), 1.0)

            def v_win(tb, h):
                # 128-wide lhsT window into the [v_even |1|1| v_odd] slot
                return v_s[:, tb, h // 2, 33 * (h % 2):33 * (h % 2) + 128]

            def emit_oproj_chunk(tb):
                for cs in range(2):
                    py = psP.tile([128, 512], f32, tag="py")
                    for jb in range(2):
                        nc.tensor.matmul(
                            py,
                            yT_s[:, jb, tb * 128:(tb + 1) * 128],
                            wp_s[:, jb, cs * 512:(cs + 1) * 512],
                            start=(jb == 0),
                            stop=(jb == 1),
                        )
                    ot = yo.tile([128, 512], bf16, tag="ot")
                    nc.vector.tensor_copy(ot, py)
                    nc.sync.dma_start(
                        out=out[tb * 128:(tb + 1) * 128, cs * 512:(cs + 1) * 512],
                        in_=ot,
                    )

            def emit_av(js, h, pend):
                nkb = 4 * js + 4
                qsl = slice(js * TS, (js + 1) * TS)
                po = psO.tile([128, 512], f32, tag="av")
                for kb, (pt, w0) in enumerate(pend):
                    nc.tensor.matmul(
                        po[:, w0:] if w0 else po,
                        v_win(kb, h),
                        pt[:, w0:] if w0 else pt,
                        start=(kb == 0),
                        stop=(kb == nkb - 1),
                        skip_group_check=True,
                    )
                # normalize y = po * (1/S): spread S across 64 partitions via a
                # DRAM bounce (DVE reciprocal is per-lane), then broadcast back.
                par = h % 2
                srow = 64 - 32 * par        # denominator row (32-aligned)
                prow = slice(par * 64, par * 64 + 64)  # y rows
                slot = h * 4 + js
                rs = rr.tile([65, TS], f32, tag="rs")
                nc.vector.tensor_copy(rs[srow:srow + 1, :], po[srow:srow + 1, :])
                nc.sync.dma_start(out=scr[slot:slot + 1, :], in_=rs[srow:srow + 1, :])
                rv = rr2.tile([64, 8], f32, tag="rv")
                nc.sync.dma_start(
                    out=rv,
                    in_=scr[slot:slot + 1, :].rearrange("a (p f) -> (a p) f", p=64),
                )
                rvr = rr2.tile([64, 8], f32, tag="rvr")
                nc.vector.reciprocal(rvr, rv)
                nc.sync.dma_start(
                    out=scr2[slot:slot + 1, :].rearrange("a (p f) -> (a p) f", p=64),
                    in_=rvr,
                )
                rb = rrb.tile([128, TS], f32, tag="rb")
                sc = scr2[slot:slot + 1, :]
                nc.gpsimd.dma_start(
                    out=rb[prow, :],
                    in_=bass.AP(tensor=sc.tensor, offset=sc.offset,
                                ap=[[0, 64]] + list(sc.ap[1:])),
                )
                nc.vector.tensor_mul(yT_s[prow, h // 2, qsl], po[prow, :], rb[prow, :])

            def phase1(ts):
                tsl = slice(ts * TS, (ts + 1) * TS)
                if ts == 0:
                    xts = xts0
                else:
                    xts = []
                    for cb in range(8):
                        xt = xq.tile([128, TS], bf16, tag="xts")
                        nc.sync.dma_start(out=xt, in_=xT[cb * 128:(cb + 1) * 128, tsl])
                        xts.append(xt)

                qkr_t = {}

                def emit_chain(jb):
                    ps = psA.tile([128, 512], f32, tag="mm")
                    for cb in range(8):
                        nc.tensor.matmul(
                            ps,
                            w1_s[:, cb, jb * 128:(jb + 1) * 128],
                            xts[cb],
                            start=(cb == 0),
                            stop=(cb == 7),
                        )
                    qkr = qkrp.tile([128, TS], bf16, tag="qkr")
                    nc.scalar.activation(qkr, ps, Ident, bias=b1_s[:, jb:jb + 1], scale=1.0)
                    qkr_t[jb] = qkr

                def emit_rope(jb):
                    psr = psA.tile([128, 512], f32, tag="mm")
                    nc.tensor.matmul(psr, pt2_s, qkr_t[jb], start=True, stop=True)
                    t1 = tp1.tile([128, TS], bf16, tag="t1")
                    nc.vector.tensor_mul(t1, qkr_t[jb], cos_s[:, tsl])
                    t2 = tp2.tile([128, TS], bf16, tag="t2")
                    nc.vector.tensor_mul(t2, psr, sin_s[:, tsl])
                    nc.vector.tensor_add(qk_s[:, jb, tsl], t1, t2)

                emit_chain(0)
                emit_chain(1)
                emit_rope(0)
                emit_chain(2)
                emit_rope(1)
                emit_chain(3)
                emit_rope(2)
                for tb2 in range(4):
                    tb = ts * 4 + tb2
                    psv = psA.tile([128, 512], f32, tag="mm")
                    for cb in range(8):
                        nc.tensor.matmul(
                            psv[:, :256],
                            xts[cb][:, tb2 * 128:(tb2 + 1) * 128],
                            wv_s[:, cb, :],
                            start=(cb == 0),
                            stop=(cb == 7),
                        )
                    # one fused copy: psum [p, pr, par, d] -> slot cols
                    # {0:64, 97:161} of both pairs (par stride 97, pair 161)
                    s0 = v_s[:, tb, 0, 0:64]
                    dst = bass.AP(tensor=s0.tensor, offset=s0.offset,
                                  ap=[s0.ap[0], [161, 2], [97, 2], [1, 64]])
                    src = psv[:, :256].rearrange("p (pr par d) -> p pr par d",
                                                 par=2, d=D)
                    nc.vector.tensor_copy(dst, src)
                emit_rope(3)

            def attention(js):
                nkb = 4 * js + 4
                pend_prev = None
                for h in range(G):
                    par = h % 2
                    hrow = slice(par * 64, par * 64 + 64)
                    qT = qk_s[hrow, h // 2, :]
                    kT = qk_s[hrow, 2 + h // 2, :]
                    pend = []
                    for kb in range(nkb):
                        roff = kb - 4 * js
                        w0 = 128 * roff if roff > 0 else 0
                        pss = psA.tile([128, 512], f32, tag="mm")
                        nc.tensor.matmul(
                            pss[:, w0:] if w0 else pss,
                            kT[:, kb * 128:(kb + 1) * 128],
                            qT[:, js * TS + w0:(js + 1) * TS],
                            start=True,
                            stop=True,
                        )
                        pt = ptp.tile([128, TS], bf16, tag="pt")
                        nc.scalar.activation(pt[:, w0:], pss[:, w0:], Exp, scale=0.125)
                        if roff >= 0:
                            nc.vector.tensor_mul(
                                pt[:, w0:w0 + 128], pt[:, w0:w0 + 128], msk_s
                            )
                        pend.append((pt, w0))
                    if h > 0:
                        emit_av(js, h - 1, pend_prev)
                        if js >= 1:
                            emit_oproj_chunk(4 * (js - 1) + (h - 1))
                    pend_prev = pend
                emit_av(js, G - 1, pend_prev)
                if js >= 1:
                    emit_oproj_chunk(4 * (js - 1) + (G - 1))

            for ts in range(NTS):
                phase1(ts)
                attention(ts)
            for tb in range(4 * (NTS - 1), NT):
                emit_oproj_chunk(tb)

    _split_multiwaits(nc)
    return nc


def _host_inputs(x, W_attn, b_attn, W_proj):
    f32 = np.float32
    import ml_dtypes

    bf16 = ml_dtypes.bfloat16

    inv = (1.0 / (10000.0 ** (np.arange(0, D, 2, dtype=f32) / f32(D)))).astype(f32)
    t = np.arange(T, dtype=f32)
    ang = np.outer(inv, t).astype(f32)            # [32, T]
    cos32, sin32 = np.cos(ang).astype(f32), np.sin(ang).astype(f32)
    cosb = np.tile(cos32, (4, 1)).astype(bf16)    # [128, T], row p -> freq p%32
    sinb = np.tile(sin32, (4, 1)).astype(bf16)

    kk = np.arange(128)[:, None]
    qq = np.arange(128)[None, :]
    msk128 = np.where(qq >= kk, f32(1), f32(0)).astype(bf16)  # [128,128]

    p64 = np.zeros((D, D), dtype=f32)
    for d in range(32):
        p64[d, d + 32] = -1.0
        p64[d + 32, d] = 1.0
    pt2 = np.zeros((128, 128), dtype=f32)
    pt2[:64, :64] = p64.T
    pt2[64:, 64:] = p64.T
    pt2 = pt2.astype(bf16)

    xTs = [np.ascontiguousarray(x[b].T).astype(bf16) for b in range(B)]

    per_g = []
    for g in range(G):
        hs = [4 * g + j for j in range(G)]
        qcols = [W_attn[:, h * D:(h + 1) * D] for h in hs]
        kcols = [W_attn[:, C + h * D:C + (h + 1) * D] for h in hs]
        qb = [b_attn[h * D:(h + 1) * D] for h in hs]
        kb_ = [b_attn[C + h * D:C + (h + 1) * D] for h in hs]
        w1 = np.concatenate(
            [qcols[0], qcols[1], qcols[2], qcols[3], kcols[0], kcols[1], kcols[2], kcols[3]],
            axis=1,
        ).astype(bf16)                             # [C, 512]: [q01 q23 k01 k23]
        b1 = np.concatenate(qb + kb_).astype(f32).reshape(4, 128).T.copy()  # [128, 4]
        wv_ = W_attn[:, 2 * C + 256 * g:2 * C + 256 * (g + 1)].astype(bf16)
        wp_ = W_proj[256 * g:256 * (g + 1), :].astype(bf16)
        per_g.append((w1, b1, wv_, wp_))

    shared = dict(cosb=cosb, sinb=sinb, msk=msk128, pt2=pt2)
    in_maps = []
    for i in range(NCORES):
        b, g = i // 4, i % 4
        w1, b1, wv_, wp_ = per_g[g]
        in_maps.append(dict(xT=xTs[b], w1=w1, b1=b1, wv=wv_, wp=wp_, **shared))
    return in_maps


def kernel(x, W_attn, b_attn, W_proj, b_proj):
    from concourse.bass_utils import run_bass_kernel_spmd

    x = np.asarray(x, dtype=np.float32)
    W_attn = np.asarray(W_attn, dtype=np.float32)
    b_attn = np.asarray(b_attn, dtype=np.float32)
    W_proj = np.asarray(W_proj, dtype=np.float32)
    b_proj = np.asarray(b_proj, dtype=np.float32)

    if "nc" not in _cached:
        _cached["nc"] = _build()
    nc = _cached["nc"]

    in_maps = _host_inputs(x, W_attn, b_attn, W_proj)
    res = run_bass_kernel_spmd(nc, in_maps, core_ids=list(range(NCORES)))
    _cached["last_results"] = res

    const = (b_proj + b_attn[2 * C:] @ W_proj).astype(np.float32)
    y = np.empty((B, T, C), dtype=np.float32)
    for b in range(B):
        acc = res.results[4 * b]["out"].astype(np.float32)
        for g in range(1, 4):
            acc = acc + res.results[4 * b + g]["out"].astype(np.float32)
        y[b] = acc + const
    return y


# revision 15
# speedup vs baseline: 1.1287x; 1.0173x over previous
"""Causal self-attention (B=2, T=2048, C=1024, H=16, RoPE) on 8 TRN2 NeuronCores.

Sharding: core i handles batch b = i//4 and head group g = i%4 (4 heads each).
Each core computes q/k (transposed, RoPE'd), v, causal attention, and a partial
output projection; the host sums the 4 partials per batch element (tensor-
parallel unshard) and adds the constant term b_proj + b_v @ W_proj, which is
independent of x because softmax rows sum to 1.

Layout strategy (no on-chip transposes):
  - host passes x^T  [C, T]
  - q^T, k^T computed as (W^T x^T) with j (head*dim) on partitions
  - rotate_half(q) computed on-chip as P @ q^T (signed permutation matmul)
  - v computed in natural [t, j] layout into per-t-block slots of a single
    v_s strip; the AV stationary operand is a two-block access pattern
    [v_head(64) | ones/pad(64)] so the same matmul also produces the softmax
    denominator (flash-style deferred normalization, no max subtraction)
  - scores computed transposed: s^T[k, q] = k^T(d,k)^T . q^T(d,q)
  - causal structure exploited at 128-column granularity: matmul/exp/AV are
    column-windowed on the diagonal q-super so fully-masked regions are never
    computed; the partial 128x128 diagonal block is masked multiplicatively
  - attention is software-pipelined at head granularity (QK chain of head h
    runs on PE while exps of head h-1 drain into its AV chain) and the output
    projection of q-super js-1 is interleaved between heads of q-super js
Matmul operands are bf16; softmax stays f32; RoPE tables and adds in bf16.
"""

import numpy as np

B, T, C, H, D = 2, 2048, 1024, 16, 64
G = 4           # heads per core
NCORES = 8
TS = 512        # t / q super-tile width
NT = T // 128   # 16 t-blocks
NTS = T // TS   # 4 t-supers

_cached = {}


def _apply_workarounds():
    """This neuronxcc build rejects TPB instructions with >1 embedded sem wait.
    Patch the Tile drain and add a BIR pass splitting extra waits into
    standalone EventSemaphore instructions on the same (in-order) engine."""
    import concourse.tile as tile
    import concourse.mybir as mybir
    from concourse.vector_clock import ScopedClock

    if getattr(tile.TileContext, "_multiwait_patched", False):
        return

    def _drain_and_barrier(self, tick_clock, wait_clock):
        nc = self.nc
        probe = nc.sync.nop(nofuse=True)
        wait_clock.add_sem_waits(probe.ins, ScopedClock({None: tick_clock.global_clock}))
        si = probe.ins.sync_info
        waits = list(si.on_wait) if si and si.on_wait else []
        if si is not None:
            si.on_wait = []
        by_num = {h.num: h for h in self.sems.allocated().values()}
        for w in waits:
            nc.sync.wait_ge(by_num[w.id], w.wait_value)
        nc.sync.drain()
        nc.all_engine_barrier()
        popped = nc._tile_sem_poison_stack.pop()
        assert popped is self._sem_poison
        nc.clear_and_free_semaphores(list(self.sems.allocated().values()))
        nc.all_engine_barrier()

    tile.TileContext._drain_and_barrier = _drain_and_barrier
    tile.TileContext._multiwait_patched = True


def _split_multiwaits(nc, maxw=1):
    import concourse.mybir as mybir

    n = 0
    for f in nc.m.functions:
        for bb in f.blocks:
            insts = list(bb.instructions)
            out = []
            changed = False
            for inst in insts:
                si = inst.sync_info
                waits = list(si.on_wait) if si and si.on_wait else []
                if len(waits) > maxw:
                    for k, w in enumerate(waits[: len(waits) - maxw]):
                        out.append(
                            mybir.InstEventSemaphore(
                                name=f"{inst.name}-xw{k}",
                                engine=inst.engine,
                                ins=[],
                                outs=[],
                                sync_info=mybir.SyncInfo(on_wait=[w], on_update=[]),
                            )
                        )
                        n += 1
                    si.on_wait = waits[len(waits) - maxw :]
                    changed = True
                out.append(inst)
            if changed:
                bb.instructions.clear()
                for i in out:
                    bb.add_instruction(i)
    return n


def _build():
    import concourse.bass as bass
    import concourse.mybir as mybir
    import concourse.tile as tile

    _apply_workarounds()

    f32 = mybir.dt.float32
    bf16 = mybir.dt.bfloat16
    Exp = mybir.ActivationFunctionType.Exp
    Ident = mybir.ActivationFunctionType.Identity

    nc = bass.Bass()

    xT = nc.dram_tensor("xT", [C, T], bf16, kind="ExternalInput")
    w1 = nc.dram_tensor("w1", [C, 512], bf16, kind="ExternalInput")     # [q01 q23 k01 k23]
    b1 = nc.dram_tensor("b1", [128, 4], f32, kind="ExternalInput")
    wv = nc.dram_tensor("wv", [C, 256], bf16, kind="ExternalInput")
    wp = nc.dram_tensor("wp", [256, C], bf16, kind="ExternalInput")
    cosb = nc.dram_tensor("cosb", [128, T], bf16, kind="ExternalInput")
    sinb = nc.dram_tensor("sinb", [128, T], bf16, kind="ExternalInput")
    msk = nc.dram_tensor("msk", [128, 128], bf16, kind="ExternalInput")
    pt2 = nc.dram_tensor("pt2", [128, 128], bf16, kind="ExternalInput")  # rotate-half perm^T
    out = nc.dram_tensor("out", [T, C], bf16, kind="ExternalOutput")
    scr = nc.dram_tensor("scr", [16, TS], f32)                          # S bounce
    scr2 = nc.dram_tensor("scr2", [16, TS], f32)                        # 1/S bounce

    with tile.TileContext(nc) as tc:
        with (
            tc.tile_pool(name="persist", bufs=1) as per,
            tc.tile_pool(name="xq", bufs=16) as xq,
            tc.tile_pool(name="qkr", bufs=3) as qkrp,
            tc.tile_pool(name="t1", bufs=2) as tp1,
            tc.tile_pool(name="t2", bufs=2) as tp2,
            tc.tile_pool(name="pt", bufs=34) as ptp,
            tc.tile_pool(name="yu", bufs=3) as yup,
            tc.tile_pool(name="rr2", bufs=3) as rr2,
            tc.tile_pool(name="rrb", bufs=2) as rrb,
            tc.tile_pool(name="yo", bufs=4) as yo,
            tc.tile_pool(name="psA", bufs=4, space="PSUM") as psA,
            tc.tile_pool(name="psO", bufs=2, space="PSUM") as psO,
            tc.tile_pool(name="psP", bufs=2, space="PSUM") as psP,
        ):
            # ---- persistent tiles ----
            w1_s = per.tile([128, 8, 512], bf16)
            wv_s = per.tile([128, 8, 256], bf16)
            wp_s = per.tile([128, 2, C], bf16)
            b1_s = per.tile([128, 4], f32)
            cos_s = per.tile([128, T], bf16)
            sin_s = per.tile([128, T], bf16)
            msk_s = per.tile([128, 128], bf16)
            pt2_s = per.tile([128, 128], bf16)
            qk_s = per.tile([128, 4, T], bf16)        # [q01' q23' k01' k23']
            # v storage per head pair:
            #   [v_even(0:64) | ones(64:66) | gap(66:97) | v_odd(97:161)]
            # The AV lhsT is a 128-wide window: even head -> cols 0..127, so y
            # lands in psum rows 0..63 with the denominator in row 64; odd
            # head -> cols 33..160, so y lands in rows 64..127 with the
            # denominator (ones col 65) in row 32. Engine ops require
            # 32-aligned partition starts, so denominator rows must be 32/64.
            v_s = per.tile([128, NT, 2, 161], bf16)
            yT_s = per.tile([128, 2, T], bf16)

            # ---- startup DMAs in priority order: the first projection chain
            # needs w1 block cb + x block cb, so interleave them ----
            xts0 = []
            for cb in range(8):
                nc.sync.dma_start(out=w1_s[:, cb, :], in_=w1[cb * 128:(cb + 1) * 128, :])
                xt = xq.tile([128, TS], bf16, tag="xts")
                nc.sync.dma_start(out=xt, in_=xT[cb * 128:(cb + 1) * 128, 0:TS])
                xts0.append(xt)
            nc.sync.dma_start(out=b1_s, in_=b1[:])
            nc.sync.dma_start(out=pt2_s, in_=pt2[:])
            nc.sync.dma_start(out=cos_s, in_=cosb[:])
            nc.sync.dma_start(out=sin_s, in_=sinb[:])
            nc.sync.dma_start(out=msk_s, in_=msk[:])
            for cb in range(8):
                nc.sync.dma_start(out=wv_s[:, cb, :], in_=wv[cb * 128:(cb + 1) * 128, :])
            for jb in range(2):
                nc.sync.dma_start(out=wp_s[:, jb, :], in_=wp[jb * 128:(jb + 1) * 128, :])
            nc.gpsimd.memset(v_s.rearrange("p a b c -> p (a b c)"), 1.0)

            def v_win(tb, h):
                # 128-wide lhsT window into the [v_even |1|1| v_odd] slot
                return v_s[:, tb, h // 2, 33 * (h % 2):33 * (h % 2) + 128]

            def emit_oproj_chunk(tb):
                for cs in range(2):
                    py = psP.tile([128, 512], f32, tag="py")
                    for jb in range(2):
                        nc.tensor.matmul(
                            py,
                            yT_s[:, jb, tb * 128:(tb + 1) * 128],
                            wp_s[:, jb, cs * 512:(cs + 1) * 512],
                            start=(jb == 0),
                            stop=(jb == 1),
                        )
                    ot = yo.tile([128, 512], bf16, tag="ot")
                    nc.vector.tensor_copy(ot, py)
                    nc.sync.dma_start(
                        out=out[tb * 128:(tb + 1) * 128, cs * 512:(cs + 1) * 512],
                        in_=ot,
                    )

            mul_queue = []

            def emit_av(js, h, pend):
                nkb = 4 * js + 4
                qsl = slice(js * TS, (js + 1) * TS)
                po = psO.tile([128, 512], f32, tag="av")
                for kb, (pt, w0) in enumerate(pend):
                    nc.tensor.matmul(
                        po[:, w0:] if w0 else po,
                        v_win(kb, h),
                        pt[:, w0:] if w0 else pt,
                        start=(kb == 0),
                        stop=(kb == nkb - 1),
                        skip_group_check=True,
                    )
                # Release the psum bank fast: one copy grabs both the
                # unnormalized y rows and the denominator row (32-aligned
                # span), then the reciprocal bounce runs from SBUF on the
                # otherwise-idle gpsimd DMA queue.  The normalize multiply is
                # deferred (mul_queue) so the Vector queue never blocks on the
                # bounce latency.
                par = h % 2
                srow = 64 - 32 * par        # denominator row (32-aligned)
                prow = slice(par * 64, par * 64 + 64)  # y rows
                slot = h * 4 + js
                yu = yup.tile([128, TS], f32, tag="yu")
                nc.vector.tensor_copy(yu, po)
                nc.gpsimd.dma_start(out=scr[slot:slot + 1, :], in_=yu[srow:srow + 1, :])
                rv = rr2.tile([64, 8], f32, tag="rv")
                nc.gpsimd.dma_start(
                    out=rv,
                    in_=scr[slot:slot + 1, :].rearrange("a (p f) -> (a p) f", p=64),
                )
                rvr = rr2.tile([64, 8], f32, tag="rvr")
                nc.vector.reciprocal(rvr, rv)
                nc.gpsimd.dma_start(
                    out=scr2[slot:slot + 1, :].rearrange("a (p f) -> (a p) f", p=64),
                    in_=rvr,
                )
                rb = rrb.tile([128, TS], f32, tag="rb")
                sc = scr2[slot:slot + 1, :]
                nc.gpsimd.dma_start(
                    out=rb[prow, :],
                    in_=bass.AP(tensor=sc.tensor, offset=sc.offset,
                                ap=[[0, 64]] + list(sc.ap[1:])),
                )

                def do_mul(tsub=None):
                    if tsub is None:
                        nc.vector.tensor_mul(yT_s[prow, h // 2, qsl],
                                             yu[prow, :], rb[prow, :])
                    else:
                        q0 = js * TS + tsub * 128
                        nc.vector.tensor_mul(
                            yT_s[prow, h // 2, q0:q0 + 128],
                            yu[prow, tsub * 128:(tsub + 1) * 128],
                            rb[prow, tsub * 128:(tsub + 1) * 128],
                        )

                mul_queue.append(do_mul)

            def phase1(ts):
                tsl = slice(ts * TS, (ts + 1) * TS)
                if ts == 0:
                    xts = xts0
                else:
                    xts = []
                    for cb in range(8):
                        xt = xq.tile([128, TS], bf16, tag="xts")
                        nc.sync.dma_start(out=xt, in_=xT[cb * 128:(cb + 1) * 128, tsl])
                        xts.append(xt)

                qkr_t = {}

                def emit_chain(jb):
                    ps = psA.tile([128, 512], f32, tag="mm")
                    for cb in range(8):
                        nc.tensor.matmul(
                            ps,
                            w1_s[:, cb, jb * 128:(jb + 1) * 128],
                            xts[cb],
                            start=(cb == 0),
                            stop=(cb == 7),
                        )
                    qkr = qkrp.tile([128, TS], bf16, tag="qkr")
                    nc.scalar.activation(qkr, ps, Ident, bias=b1_s[:, jb:jb + 1], scale=1.0)
                    qkr_t[jb] = qkr

                def emit_rope(jb):
                    psr = psA.tile([128, 512], f32, tag="mm")
                    nc.tensor.matmul(psr, pt2_s, qkr_t[jb], start=True, stop=True)
                    t1 = tp1.tile([128, TS], bf16, tag="t1")
                    nc.vector.tensor_mul(t1, qkr_t[jb], cos_s[:, tsl])
                    t2 = tp2.tile([128, TS], bf16, tag="t2")
                    nc.vector.tensor_mul(t2, psr, sin_s[:, tsl])
                    nc.vector.tensor_add(qk_s[:, jb, tsl], t1, t2)

                emit_chain(0)
                emit_chain(1)
                emit_rope(0)
                emit_chain(2)
                emit_rope(1)
                emit_chain(3)
                emit_rope(2)
                for tb2 in range(4):
                    tb = ts * 4 + tb2
                    psv = psA.tile([128, 512], f32, tag="mm")
                    for cb in range(8):
                        nc.tensor.matmul(
                            psv[:, :256],
                            xts[cb][:, tb2 * 128:(tb2 + 1) * 128],
                            wv_s[:, cb, :],
                            start=(cb == 0),
                            stop=(cb == 7),
                        )
                    # one fused copy: psum [p, pr, par, d] -> slot cols
                    # {0:64, 97:161} of both pairs (par stride 97, pair 161)
                    s0 = v_s[:, tb, 0, 0:64]
                    dst = bass.AP(tensor=s0.tensor, offset=s0.offset,
                                  ap=[s0.ap[0], [161, 2], [97, 2], [1, 64]])
                    src = psv[:, :256].rearrange("p (pr par d) -> p pr par d",
                                                 par=2, d=D)
                    nc.vector.tensor_copy(dst, src)
                emit_rope(3)

            def attention(js):
                nkb = 4 * js + 4
                pend_prev = None
                # yT of the previous super must be fully written before this
                # super's oproj chunks (emitted below) read it
                while mul_queue:
                    mul_queue.pop(0)()
                for h in range(G):
                    par = h % 2
                    hrow = slice(par * 64, par * 64 + 64)
                    qT = qk_s[hrow, h // 2, :]
                    kT = qk_s[hrow, 2 + h // 2, :]
                    pend = []
                    masks = []
                    for kb in range(nkb):
                        roff = kb - 4 * js
                        w0 = 128 * roff if roff > 0 else 0
                        pss = psA.tile([128, 512], f32, tag="mm")
                        nc.tensor.matmul(
                            pss[:, w0:] if w0 else pss,
                            kT[:, kb * 128:(kb + 1) * 128],
                            qT[:, js * TS + w0:(js + 1) * TS],
                            start=True,
                            stop=True,
                        )
                        pt = ptp.tile([128, TS], bf16, tag="pt")
                        nc.scalar.activation(pt[:, w0:], pss[:, w0:], Exp, scale=0.125)
                        if roff >= 0:
                            masks.append((pt, w0))
                        pend.append((pt, w0))
                    if h > 0:
                        emit_av(js, h - 1, pend_prev)
                    if len(mul_queue) > 1:
                        mul_queue.pop(0)()
                    for pt, w0 in masks:
                        nc.vector.tensor_mul(
                            pt[:, w0:w0 + 128], pt[:, w0:w0 + 128], msk_s
                        )
                    if js >= 1:
                        emit_oproj_chunk(4 * (js - 1) + h)
                    pend_prev = pend
                emit_av(js, G - 1, pend_prev)

            for ts in range(NTS):
                phase1(ts)
                attention(ts)
            # tail: flush deferred normalize muls; the last head's mul is
            # split per 128-t-block so the final output projection overlaps it
            last = mul_queue.pop()
            for m in mul_queue:
                m()
            mul_queue.clear()
            for i, tb in enumerate(range(4 * (NTS - 1), NT)):
                last(i)
                emit_oproj_chunk(tb)

    _split_multiwaits(nc)
    return nc


def _host_inputs(x, W_attn, b_attn, W_proj):
    f32 = np.float32
    import ml_dtypes

    bf16 = ml_dtypes.bfloat16

    inv = (1.0 / (10000.0 ** (np.arange(0, D, 2, dtype=f32) / f32(D)))).astype(f32)
    t = np.arange(T, dtype=f32)
    ang = np.outer(inv, t).astype(f32)            # [32, T]
    cos32, sin32 = np.cos(ang).astype(f32), np.sin(ang).astype(f32)
    cosb = np.tile(cos32, (4, 1)).astype(bf16)    # [128, T], row p -> freq p%32
    sinb = np.tile(sin32, (4, 1)).astype(bf16)

    kk = np.arange(128)[:, None]
    qq = np.arange(128)[None, :]
    msk128 = np.where(qq >= kk, f32(1), f32(0)).astype(bf16)  # [128,128]

    p64 = np.zeros((D, D), dtype=f32)
    for d in range(32):
        p64[d, d + 32] = -1.0
        p64[d + 32, d] = 1.0
    pt2 = np.zeros((128, 128), dtype=f32)
    pt2[:64, :64] = p64.T
    pt2[64:, 64:] = p64.T
    pt2 = pt2.astype(bf16)

    xTs = [np.ascontiguousarray(x[b].T).astype(bf16) for b in range(B)]

    per_g = []
    for g in range(G):
        hs = [4 * g + j for j in range(G)]
        qcols = [W_attn[:, h * D:(h + 1) * D] for h in hs]
        kcols = [W_attn[:, C + h * D:C + (h + 1) * D] for h in hs]
        qb = [b_attn[h * D:(h + 1) * D] for h in hs]
        kb_ = [b_attn[C + h * D:C + (h + 1) * D] for h in hs]
        w1 = np.concatenate(
            [qcols[0], qcols[1], qcols[2], qcols[3], kcols[0], kcols[1], kcols[2], kcols[3]],
            axis=1,
        ).astype(bf16)                             # [C, 512]: [q01 q23 k01 k23]
        b1 = np.concatenate(qb + kb_).astype(f32).reshape(4, 128).T.copy()  # [128, 4]
        wv_ = W_attn[:, 2 * C + 256 * g:2 * C + 256 * (g + 1)].astype(bf16)
        wp_ = W_proj[256 * g:256 * (g + 1), :].astype(bf16)
        per_g.append((w1, b1, wv_, wp_))

    shared = dict(cosb=cosb, sinb=sinb, msk=msk128, pt2=pt2)
    in_maps = []
    for i in range(NCORES):
        b, g = i // 4, i % 4
        w1, b1, wv_, wp_ = per_g[g]
        in_maps.append(dict(xT=xTs[b], w1=w1, b1=b1, wv=wv_, wp=wp_, **shared))
    return in_maps


def kernel(x, W_attn, b_attn, W_proj, b_proj):
    from concourse.bass_utils import run_bass_kernel_spmd

    x = np.asarray(x, dtype=np.float32)
    W_attn = np.asarray(W_attn, dtype=np.float32)
    b_attn = np.asarray(b_attn, dtype=np.float32)
    W_proj = np.asarray(W_proj, dtype=np.float32)
    b_proj = np.asarray(b_proj, dtype=np.float32)

    if "nc" not in _cached:
        _cached["nc"] = _build()
    nc = _cached["nc"]

    in_maps = _host_inputs(x, W_attn, b_attn, W_proj)
    res = run_bass_kernel_spmd(nc, in_maps, core_ids=list(range(NCORES)))
    _cached["last_results"] = res

    const = (b_proj + b_attn[2 * C:] @ W_proj).astype(np.float32)
    y = np.empty((B, T, C), dtype=np.float32)
    for b in range(B):
        acc = res.results[4 * b]["out"].astype(np.float32)
        for g in range(1, 4):
            acc = acc + res.results[4 * b + g]["out"].astype(np.float32)
        y[b] = acc + const
    return y


# revision 17
# speedup vs baseline: 1.1789x; 1.0444x over previous
"""Causal self-attention (B=2, T=2048, C=1024, H=16, RoPE) on 8 TRN2 NeuronCores.

Sharding: core i handles batch b = i//4 and head group g = i%4 (4 heads each).
Each core computes q/k (transposed, RoPE'd), v, causal attention, and a partial
output projection; the host sums the 4 partials per batch element (tensor-
parallel unshard) and adds the constant term b_proj + b_v @ W_proj, which is
independent of x because softmax rows sum to 1.

Layout strategy (no on-chip transposes):
  - host passes x^T  [C, T]
  - q^T, k^T computed as (W^T x^T) with j (head*dim) on partitions
  - rotate_half(q) computed on-chip as P @ q^T (signed permutation matmul)
  - v computed in natural [t, j] layout into per-t-block slots of a single
    v_s strip; the AV stationary operand is a two-block access pattern
    [v_head(64) | ones/pad(64)] so the same matmul also produces the softmax
    denominator (flash-style deferred normalization, no max subtraction)
  - scores computed transposed: s^T[k, q] = k^T(d,k)^T . q^T(d,q)
  - causal structure exploited at 128-column granularity: matmul/exp/AV are
    column-windowed on the diagonal q-super so fully-masked regions are never
    computed; the partial 128x128 diagonal block is masked multiplicatively
  - attention is software-pipelined at head granularity (QK chain of head h
    runs on PE while exps of head h-1 drain into its AV chain) and the output
    projection of q-super js-1 is interleaved between heads of q-super js
Matmul operands are bf16; softmax stays f32; RoPE tables and adds in bf16.
"""

import numpy as np

B, T, C, H, D = 2, 2048, 1024, 16, 64
G = 4           # heads per core
NCORES = 8
TS = 512        # t / q super-tile width
NT = T // 128   # 16 t-blocks
NTS = T // TS   # 4 t-supers

_cached = {}


def _apply_workarounds():
    """This neuronxcc build rejects TPB instructions with >1 embedded sem wait.
    Patch the Tile drain and add a BIR pass splitting extra waits into
    standalone EventSemaphore instructions on the same (in-order) engine."""
    import concourse.tile as tile
    import concourse.mybir as mybir
    from concourse.vector_clock import ScopedClock

    if getattr(tile.TileContext, "_multiwait_patched", False):
        return

    def _drain_and_barrier(self, tick_clock, wait_clock):
        nc = self.nc
        probe = nc.sync.nop(nofuse=True)
        wait_clock.add_sem_waits(probe.ins, ScopedClock({None: tick_clock.global_clock}))
        si = probe.ins.sync_info
        waits = list(si.on_wait) if si and si.on_wait else []
        if si is not None:
            si.on_wait = []
        by_num = {h.num: h for h in self.sems.allocated().values()}
        for w in waits:
            nc.sync.wait_ge(by_num[w.id], w.wait_value)
        nc.sync.drain()
        nc.all_engine_barrier()
        popped = nc._tile_sem_poison_stack.pop()
        assert popped is self._sem_poison
        nc.clear_and_free_semaphores(list(self.sems.allocated().values()))
        nc.all_engine_barrier()

    tile.TileContext._drain_and_barrier = _drain_and_barrier
    tile.TileContext._multiwait_patched = True


def _split_multiwaits(nc, maxw=1):
    import concourse.mybir as mybir

    n = 0
    for f in nc.m.functions:
        for bb in f.blocks:
            insts = list(bb.instructions)
            out = []
            changed = False
            for inst in insts:
                si = inst.sync_info
                waits = list(si.on_wait) if si and si.on_wait else []
                if len(waits) > maxw:
                    for k, w in enumerate(waits[: len(waits) - maxw]):
                        out.append(
                            mybir.InstEventSemaphore(
                                name=f"{inst.name}-xw{k}",
                                engine=inst.engine,
                                ins=[],
                                outs=[],
                                sync_info=mybir.SyncInfo(on_wait=[w], on_update=[]),
                            )
                        )
                        n += 1
                    si.on_wait = waits[len(waits) - maxw :]
                    changed = True
                out.append(inst)
            if changed:
                bb.instructions.clear()
                for i in out:
                    bb.add_instruction(i)
    return n


def _build():
    import concourse.bass as bass
    import concourse.mybir as mybir
    import concourse.tile as tile

    _apply_workarounds()

    f32 = mybir.dt.float32
    bf16 = mybir.dt.bfloat16
    Exp = mybir.ActivationFunctionType.Exp
    Ident = mybir.ActivationFunctionType.Identity

    nc = bass.Bass()

    xT = nc.dram_tensor("xT", [C, T], bf16, kind="ExternalInput")
    w1 = nc.dram_tensor("w1", [C, 512], bf16, kind="ExternalInput")     # [q01 q23 k01 k23]
    b1 = nc.dram_tensor("b1", [128, 4], f32, kind="ExternalInput")
    wv = nc.dram_tensor("wv", [C, 256], bf16, kind="ExternalInput")
    wp = nc.dram_tensor("wp", [256, C], bf16, kind="ExternalInput")
    cosb = nc.dram_tensor("cosb", [128, T], bf16, kind="ExternalInput")
    sinb = nc.dram_tensor("sinb", [128, T], bf16, kind="ExternalInput")
    msk = nc.dram_tensor("msk", [128, 128], bf16, kind="ExternalInput")
    pt2 = nc.dram_tensor("pt2", [128, 128], bf16, kind="ExternalInput")  # rotate-half perm^T
    out = nc.dram_tensor("out", [T, C], bf16, kind="ExternalOutput")
    scr = nc.dram_tensor("scr", [16, TS], f32)                          # S bounce
    scr2 = nc.dram_tensor("scr2", [16, TS], f32)                        # 1/S bounce

    with tile.TileContext(nc) as tc:
        with (
            tc.tile_pool(name="persist", bufs=1) as per,
            tc.tile_pool(name="xq", bufs=16) as xq,
            tc.tile_pool(name="qkr", bufs=3) as qkrp,
            tc.tile_pool(name="t1", bufs=2) as tp1,
            tc.tile_pool(name="t2", bufs=2) as tp2,
            tc.tile_pool(name="pt", bufs=34) as ptp,
            tc.tile_pool(name="yu", bufs=3) as yup,
            tc.tile_pool(name="rr2", bufs=3) as rr2,
            tc.tile_pool(name="rrb", bufs=2) as rrb,
            tc.tile_pool(name="yo", bufs=4) as yo,
            tc.tile_pool(name="psA", bufs=4, space="PSUM") as psA,
            tc.tile_pool(name="psO", bufs=2, space="PSUM") as psO,
            tc.tile_pool(name="psP", bufs=2, space="PSUM") as psP,
        ):
            # ---- persistent tiles ----
            w1_s = per.tile([128, 8, 512], bf16)
            wv_s = per.tile([128, 8, 256], bf16)
            wp_s = per.tile([128, 2, C], bf16)
            b1_s = per.tile([128, 4], f32)
            cos_s = per.tile([128, T], bf16)
            sin_s = per.tile([128, T], bf16)
            msk_s = per.tile([128, 128], bf16)
            pt2_s = per.tile([128, 128], bf16)
            qk_s = per.tile([128, 4, T], bf16)        # [q01' q23' k01' k23']
            # v storage per head pair:
            #   [v_even(0:64) | ones(64:66) | gap(66:97) | v_odd(97:161)]
            # The AV lhsT is a 128-wide window: even head -> cols 0..127, so y
            # lands in psum rows 0..63 with the denominator in row 64; odd
            # head -> cols 33..160, so y lands in rows 64..127 with the
            # denominator (ones col 65) in row 32. Engine ops require
            # 32-aligned partition starts, so denominator rows must be 32/64.
            v_s = per.tile([128, NT, 2, 161], bf16)
            yT_s = per.tile([128, 2, T], bf16)

            # ---- startup DMAs in priority order: the first projection chain
            # needs w1 block cb + x block cb, so interleave them ----
            xts0 = []
            for cb in range(8):
                nc.sync.dma_start(out=w1_s[:, cb, :], in_=w1[cb * 128:(cb + 1) * 128, :])
                xt = xq.tile([128, TS], bf16, tag="xts")
                nc.sync.dma_start(out=xt, in_=xT[cb * 128:(cb + 1) * 128, 0:TS])
                xts0.append(xt)
            nc.sync.dma_start(out=b1_s, in_=b1[:])
            nc.sync.dma_start(out=pt2_s, in_=pt2[:])
            nc.sync.dma_start(out=cos_s, in_=cosb[:])
            nc.sync.dma_start(out=sin_s, in_=sinb[:])
            nc.sync.dma_start(out=msk_s, in_=msk[:])
            for cb in range(8):
                nc.sync.dma_start(out=wv_s[:, cb, :], in_=wv[cb * 128:(cb + 1) * 128, :])
            for jb in range(2):
                nc.sync.dma_start(out=wp_s[:, jb, :], in_=wp[jb * 128:(jb + 1) * 128, :])
            nc.gpsimd.memset(v_s.rearrange("p a b c -> p (a b c)"), 1.0)

            def v_win(tb, h):
                # 128-wide lhsT window into the [v_even |1|1| v_odd] slot
                return v_s[:, tb, h // 2, 33 * (h % 2):33 * (h % 2) + 128]

            def emit_oproj_chunk(tb):
                for cs in range(2):
                    py = psP.tile([128, 512], f32, tag="py")
                    for jb in range(2):
                        nc.tensor.matmul(
                            py,
                            yT_s[:, jb, tb * 128:(tb + 1) * 128],
                            wp_s[:, jb, cs * 512:(cs + 1) * 512],
                            start=(jb == 0),
                            stop=(jb == 1),
                        )
                    ot = yo.tile([128, 512], bf16, tag="ot")
                    nc.vector.tensor_copy(ot, py)
                    nc.sync.dma_start(
                        out=out[tb * 128:(tb + 1) * 128, cs * 512:(cs + 1) * 512],
                        in_=ot,
                    )

            mul_queue = []

            def emit_av(js, h, pend):
                nkb = 4 * js + 4
                qsl = slice(js * TS, (js + 1) * TS)
                po = psO.tile([128, 512], f32, tag="av")
                for kb, (pt, w0) in enumerate(pend):
                    nc.tensor.matmul(
                        po[:, w0:] if w0 else po,
                        v_win(kb, h),
                        pt[:, w0:] if w0 else pt,
                        start=(kb == 0),
                        stop=(kb == nkb - 1),
                        skip_group_check=True,
                    )
                # Release the psum bank fast: one copy grabs both the
                # unnormalized y rows and the denominator row (32-aligned
                # span), then the reciprocal bounce runs from SBUF on the
                # otherwise-idle gpsimd DMA queue.  The normalize multiply is
                # deferred (mul_queue) so the Vector queue never blocks on the
                # bounce latency.
                par = h % 2
                srow = 64 - 32 * par        # denominator row (32-aligned)
                prow = slice(par * 64, par * 64 + 64)  # y rows
                slot = h * 4 + js
                yu = yup.tile([128, TS], f32, tag="yu")
                nc.vector.tensor_copy(yu, po)
                nc.sync.dma_start(out=scr[slot:slot + 1, :], in_=yu[srow:srow + 1, :])
                rv = rr2.tile([64, 8], f32, tag="rv")
                nc.sync.dma_start(
                    out=rv,
                    in_=scr[slot:slot + 1, :].rearrange("a (p f) -> (a p) f", p=64),
                )
                rvr = rr2.tile([64, 8], f32, tag="rvr")
                nc.vector.reciprocal(rvr, rv)
                nc.sync.dma_start(
                    out=scr2[slot:slot + 1, :].rearrange("a (p f) -> (a p) f", p=64),
                    in_=rvr,
                )
                rb = rrb.tile([128, TS], f32, tag="rb")
                sc = scr2[slot:slot + 1, :]
                nc.gpsimd.dma_start(
                    out=rb[prow, :],
                    in_=bass.AP(tensor=sc.tensor, offset=sc.offset,
                                ap=[[0, 64]] + list(sc.ap[1:])),
                )

                def do_mul(tsub=None):
                    if tsub is None:
                        nc.vector.tensor_mul(yT_s[prow, h // 2, qsl],
                                             yu[prow, :], rb[prow, :])
                    else:
                        q0 = js * TS + tsub * 128
                        nc.vector.tensor_mul(
                            yT_s[prow, h // 2, q0:q0 + 128],
                            yu[prow, tsub * 128:(tsub + 1) * 128],
                            rb[prow, tsub * 128:(tsub + 1) * 128],
                        )

                mul_queue.append(do_mul)

            def phase1(ts):
                tsl = slice(ts * TS, (ts + 1) * TS)
                if ts == 0:
                    xts = xts0
                else:
                    xts = []
                    for cb in range(8):
                        xt = xq.tile([128, TS], bf16, tag="xts")
                        nc.sync.dma_start(out=xt, in_=xT[cb * 128:(cb + 1) * 128, tsl])
                        xts.append(xt)

                qkr_t = {}

                def emit_chain(jb):
                    ps = psA.tile([128, 512], f32, tag="mm")
                    for cb in range(8):
                        nc.tensor.matmul(
                            ps,
                            w1_s[:, cb, jb * 128:(jb + 1) * 128],
                            xts[cb],
                            start=(cb == 0),
                            stop=(cb == 7),
                        )
                    qkr = qkrp.tile([128, TS], bf16, tag="qkr")
                    nc.scalar.activation(qkr, ps, Ident, bias=b1_s[:, jb:jb + 1], scale=1.0)
                    qkr_t[jb] = qkr

                def emit_rope(jb):
                    psr = psA.tile([128, 512], f32, tag="mm")
                    nc.tensor.matmul(psr, pt2_s, qkr_t[jb], start=True, stop=True)
                    t1 = tp1.tile([128, TS], bf16, tag="t1")
                    nc.vector.tensor_mul(t1, qkr_t[jb], cos_s[:, tsl])
                    t2 = tp2.tile([128, TS], bf16, tag="t2")
                    nc.vector.tensor_mul(t2, psr, sin_s[:, tsl])
                    nc.vector.tensor_add(qk_s[:, jb, tsl], t1, t2)

                emit_chain(0)
                emit_chain(1)
                emit_rope(0)
                emit_chain(2)
                emit_rope(1)
                emit_chain(3)
                emit_rope(2)
                for tb2 in range(4):
                    tb = ts * 4 + tb2
                    psv = psA.tile([128, 512], f32, tag="mm")
                    for cb in range(8):
                        nc.tensor.matmul(
                            psv[:, :256],
                            xts[cb][:, tb2 * 128:(tb2 + 1) * 128],
                            wv_s[:, cb, :],
                            start=(cb == 0),
                            stop=(cb == 7),
                        )
                    # one fused copy: psum [p, pr, par, d] -> slot cols
                    # {0:64, 97:161} of both pairs (par stride 97, pair 161)
                    s0 = v_s[:, tb, 0, 0:64]
                    dst = bass.AP(tensor=s0.tensor, offset=s0.offset,
                                  ap=[s0.ap[0], [161, 2], [97, 2], [1, 64]])
                    src = psv[:, :256].rearrange("p (pr par d) -> p pr par d",
                                                 par=2, d=D)
                    nc.vector.tensor_copy(dst, src)
                emit_rope(3)

            def attention(js):
                nkb = 4 * js + 4
                pend_prev = None
                # yT of the previous super must be fully written before this
                # super's oproj chunks (emitted below) read it
                while mul_queue:
                    mul_queue.pop(0)()
                for h in range(G):
                    par = h % 2
                    hrow = slice(par * 64, par * 64 + 64)
                    qT = qk_s[hrow, h // 2, :]
                    kT = qk_s[hrow, 2 + h // 2, :]
                    pend = []
                    masks = []
                    for kb in range(nkb):
                        roff = kb - 4 * js
                        w0 = 128 * roff if roff > 0 else 0
                        pss = psA.tile([128, 512], f32, tag="mm")
                        nc.tensor.matmul(
                            pss[:, w0:] if w0 else pss,
                            kT[:, kb * 128:(kb + 1) * 128],
                            qT[:, js * TS + w0:(js + 1) * TS],
                            start=True,
                            stop=True,
                        )
                        pt = ptp.tile([128, TS], bf16, tag="pt")
                        nc.scalar.activation(pt[:, w0:], pss[:, w0:], Exp, scale=0.125)
                        if roff >= 0:
                            masks.append((pt, w0))
                        pend.append((pt, w0))
                    if h > 0:
                        emit_av(js, h - 1, pend_prev)
                    if len(mul_queue) > 1:
                        mul_queue.pop(0)()
                    for pt, w0 in masks:
                        nc.vector.tensor_mul(
                            pt[:, w0:w0 + 128], pt[:, w0:w0 + 128], msk_s
                        )
                    if js >= 1:
                        emit_oproj_chunk(4 * (js - 1) + h)
                    pend_prev = pend
                emit_av(js, G - 1, pend_prev)

            # attention(js) is emitted after phase1(js+1): the projection
            # matmuls give Scalar a head start on the attention exps, so the
            # AV chains never wait on the exp stream
            phase1(0)
            for ts in range(1, NTS):
                phase1(ts)
                attention(ts - 1)
            attention(NTS - 1)
            # tail: flush deferred normalize muls; the last head's mul is
            # split per 128-t-block so the final output projection overlaps it
            last = mul_queue.pop()
            for m in mul_queue:
                m()
            mul_queue.clear()
            # keep the PE p-state hot across the final reciprocal-bounce wait
            for w in range(10):
                pw = psA.tile([128, 512], f32, tag="mm")
                nc.tensor.matmul(pw, pt2_s, qk_s[:, 0, 0:TS], start=True, stop=True)
            for i, tb in enumerate(range(4 * (NTS - 1), NT)):
                last(i)
                emit_oproj_chunk(tb)

    _split_multiwaits(nc)
    return nc


def _host_inputs(x, W_attn, b_attn, W_proj):
    f32 = np.float32
    import ml_dtypes

    bf16 = ml_dtypes.bfloat16

    inv = (1.0 / (10000.0 ** (np.arange(0, D, 2, dtype=f32) / f32(D)))).astype(f32)
    t = np.arange(T, dtype=f32)
    ang = np.outer(inv, t).astype(f32)            # [32, T]
    cos32, sin32 = np.cos(ang).astype(f32), np.sin(ang).astype(f32)
    cosb = np.tile(cos32, (4, 1)).astype(bf16)    # [128, T], row p -> freq p%32
    sinb = np.tile(sin32, (4, 1)).astype(bf16)

    kk = np.arange(128)[:, None]
    qq = np.arange(128)[None, :]
    msk128 = np.where(qq >= kk, f32(1), f32(0)).astype(bf16)  # [128,128]

    p64 = np.zeros((D, D), dtype=f32)
    for d in range(32):
        p64[d, d + 32] = -1.0
        p64[d + 32, d] = 1.0
    pt2 = np.zeros((128, 128), dtype=f32)
    pt2[:64, :64] = p64.T
    pt2[64:, 64:] = p64.T
    pt2 = pt2.astype(bf16)

    xTs = [np.ascontiguousarray(x[b].T).astype(bf16) for b in range(B)]

    per_g = []
    for g in range(G):
        hs = [4 * g + j for j in range(G)]
        qcols = [W_attn[:, h * D:(h + 1) * D] for h in hs]
        kcols = [W_attn[:, C + h * D:C + (h + 1) * D] for h in hs]
        qb = [b_attn[h * D:(h + 1) * D] for h in hs]
        kb_ = [b_attn[C + h * D:C + (h + 1) * D] for h in hs]
        w1 = np.concatenate(
            [qcols[0], qcols[1], qcols[2], qcols[3], kcols[0], kcols[1], kcols[2], kcols[3]],
            axis=1,
        ).astype(bf16)                             # [C, 512]: [q01 q23 k01 k23]
        b1 = np.concatenate(qb + kb_).astype(f32).reshape(4, 128).T.copy()  # [128, 4]
        wv_ = W_attn[:, 2 * C + 256 * g:2 * C + 256 * (g + 1)].astype(bf16)
        wp_ = W_proj[256 * g:256 * (g + 1), :].astype(bf16)
        per_g.append((w1, b1, wv_, wp_))

    shared = dict(cosb=cosb, sinb=sinb, msk=msk128, pt2=pt2)
    in_maps = []
    for i in range(NCORES):
        b, g = i // 4, i % 4
        w1, b1, wv_, wp_ = per_g[g]
        in_maps.append(dict(xT=xTs[b], w1=w1, b1=b1, wv=wv_, wp=wp_, **shared))
    return in_maps


def kernel(x, W_attn, b_attn, W_proj, b_proj):
    from concourse.bass_utils import run_bass_kernel_spmd

    x = np.asarray(x, dtype=np.float32)
    W_attn = np.asarray(W_attn, dtype=np.float32)
    b_attn = np.asarray(b_attn, dtype=np.float32)
    W_proj = np.asarray(W_proj, dtype=np.float32)
    b_proj = np.asarray(b_proj, dtype=np.float32)

    if "nc" not in _cached:
        _cached["nc"] = _build()
    nc = _cached["nc"]

    in_maps = _host_inputs(x, W_attn, b_attn, W_proj)
    res = run_bass_kernel_spmd(nc, in_maps, core_ids=list(range(NCORES)))
    _cached["last_results"] = res

    const = (b_proj + b_attn[2 * C:] @ W_proj).astype(np.float32)
    y = np.empty((B, T, C), dtype=np.float32)
    for b in range(B):
        acc = res.results[4 * b]["out"].astype(np.float32)
        for g in range(1, 4):
            acc = acc + res.results[4 * b + g]["out"].astype(np.float32)
        y[b] = acc + const
    return y


# revision 29
# speedup vs baseline: 1.2207x; 1.0355x over previous
"""Causal self-attention (B=2, T=2048, C=1024, H=16, RoPE) on 8 TRN2 NeuronCores.

Sharding: core i handles batch b = i//4 and head group g = i%4 (4 heads each).
Each core computes q/k (transposed, RoPE'd), v, causal attention, and a partial
output projection; the host sums the 4 partials per batch element (tensor-
parallel unshard) and adds the constant term b_proj + b_v @ W_proj, which is
independent of x because softmax rows sum to 1.

Layout strategy (no on-chip transposes):
  - host passes x^T  [C, T]
  - q^T, k^T computed as (W^T x^T) with j (head*dim) on partitions
  - rotate_half(q) computed on-chip as P @ q^T (signed permutation matmul)
  - v computed in natural [t, j] layout into per-t-block slots of a single
    v_s strip; the AV stationary operand is a two-block access pattern
    [v_head(64) | ones/pad(64)] so the same matmul also produces the softmax
    denominator (flash-style deferred normalization, no max subtraction)
  - scores computed transposed: s^T[k, q] = k^T(d,k)^T . q^T(d,q)
  - causal structure exploited at 128-column granularity: matmul/exp/AV are
    column-windowed on the diagonal q-super so fully-masked regions are never
    computed; the partial 128x128 diagonal block is masked multiplicatively
  - attention is software-pipelined at head granularity (QK chain of head h
    runs on PE while exps of head h-1 drain into its AV chain) and the output
    projection of q-super js-1 is interleaved between heads of q-super js
Matmul operands are bf16; softmax stays f32; RoPE tables and adds in bf16.
"""

import numpy as np

B, T, C, H, D = 2, 2048, 1024, 16, 64
G = 4           # heads per core
NCORES = 8
TS = 512        # t / q super-tile width
NT = T // 128   # 16 t-blocks
NTS = T // TS   # 4 t-supers

_cached = {}


def _apply_workarounds():
    """This neuronxcc build rejects TPB instructions with >1 embedded sem wait.
    Patch the Tile drain and add a BIR pass splitting extra waits into
    standalone EventSemaphore instructions on the same (in-order) engine."""
    import concourse.tile as tile
    import concourse.mybir as mybir
    from concourse.vector_clock import ScopedClock

    if getattr(tile.TileContext, "_multiwait_patched", False):
        return

    def _drain_and_barrier(self, tick_clock, wait_clock):
        nc = self.nc
        probe = nc.sync.nop(nofuse=True)
        wait_clock.add_sem_waits(probe.ins, ScopedClock({None: tick_clock.global_clock}))
        si = probe.ins.sync_info
        waits = list(si.on_wait) if si and si.on_wait else []
        if si is not None:
            si.on_wait = []
        by_num = {h.num: h for h in self.sems.allocated().values()}
        for w in waits:
            nc.sync.wait_ge(by_num[w.id], w.wait_value)
        nc.sync.drain()
        nc.all_engine_barrier()
        popped = nc._tile_sem_poison_stack.pop()
        assert popped is self._sem_poison
        nc.clear_and_free_semaphores(list(self.sems.allocated().values()))
        nc.all_engine_barrier()

    tile.TileContext._drain_and_barrier = _drain_and_barrier
    tile.TileContext._multiwait_patched = True


def _split_multiwaits(nc, maxw=1):
    import concourse.mybir as mybir

    n = 0
    for f in nc.m.functions:
        for bb in f.blocks:
            insts = list(bb.instructions)
            out = []
            changed = False
            for inst in insts:
                si = inst.sync_info
                waits = list(si.on_wait) if si and si.on_wait else []
                if len(waits) > maxw:
                    for k, w in enumerate(waits[: len(waits) - maxw]):
                        out.append(
                            mybir.InstEventSemaphore(
                                name=f"{inst.name}-xw{k}",
                                engine=inst.engine,
                                ins=[],
                                outs=[],
                                sync_info=mybir.SyncInfo(on_wait=[w], on_update=[]),
                            )
                        )
                        n += 1
                    si.on_wait = waits[len(waits) - maxw :]
                    changed = True
                out.append(inst)
            if changed:
                bb.instructions.clear()
                for i in out:
                    bb.add_instruction(i)
    return n


def _build():
    import concourse.bass as bass
    import concourse.mybir as mybir
    import concourse.tile as tile

    _apply_workarounds()

    f32 = mybir.dt.float32
    bf16 = mybir.dt.bfloat16
    Exp = mybir.ActivationFunctionType.Exp
    Ident = mybir.ActivationFunctionType.Identity

    nc = bass.Bass()

    xT = nc.dram_tensor("xT", [C, T], bf16, kind="ExternalInput")
    w1 = nc.dram_tensor("w1", [C, 512], bf16, kind="ExternalInput")     # [q01 q23 k01 k23]
    b1 = nc.dram_tensor("b1", [128, 4], f32, kind="ExternalInput")
    wv = nc.dram_tensor("wv", [C, 256], bf16, kind="ExternalInput")
    wp = nc.dram_tensor("wp", [256, C], bf16, kind="ExternalInput")
    cosb = nc.dram_tensor("cosb", [128, T], bf16, kind="ExternalInput")
    sinb = nc.dram_tensor("sinb", [128, T], bf16, kind="ExternalInput")
    msk = nc.dram_tensor("msk", [128, 128], bf16, kind="ExternalInput")
    pt2 = nc.dram_tensor("pt2", [128, 128], bf16, kind="ExternalInput")  # rotate-half perm^T
    idn = nc.dram_tensor("idn", [128, 128], f32, kind="ExternalInput")
    out = nc.dram_tensor("out", [T, C], bf16, kind="ExternalOutput")
    scr = nc.dram_tensor("scr", [16, TS], f32)                          # S bounce
    scr2 = nc.dram_tensor("scr2", [16, TS], f32)                        # 1/S bounce

    with tile.TileContext(nc) as tc:
        with (
            tc.tile_pool(name="persist", bufs=1) as per,
            tc.tile_pool(name="xq", bufs=16) as xq,
            tc.tile_pool(name="qkr", bufs=3) as qkrp,
            tc.tile_pool(name="t1", bufs=2) as tp1,
            tc.tile_pool(name="t2", bufs=2) as tp2,
            tc.tile_pool(name="pt", bufs=34) as ptp,
            tc.tile_pool(name="yu", bufs=3) as yup,
            tc.tile_pool(name="rr2", bufs=3) as rr2,
            tc.tile_pool(name="rrb", bufs=2) as rrb,
            tc.tile_pool(name="yo", bufs=4) as yo,
            tc.tile_pool(name="psA", bufs=4, space="PSUM") as psA,
            tc.tile_pool(name="psO", bufs=2, space="PSUM") as psO,
            tc.tile_pool(name="psP", bufs=2, space="PSUM") as psP,
        ):
            # ---- persistent tiles ----
            w1_s = per.tile([128, 8, 512], bf16)
            wv_s = per.tile([128, 8, 256], bf16)
            wp_s = per.tile([128, 2, C], bf16)
            b1_s = per.tile([128, 4], f32)
            cos_s = per.tile([128, T], bf16)
            sin_s = per.tile([128, T], bf16)
            msk_s = per.tile([128, 128], bf16)
            pt2_s = per.tile([128, 128], bf16)
            ones_s = per.tile([64, 128], f32)
            idn_s = per.tile([128, 128], f32)
            qk_s = per.tile([128, 4, T], bf16)        # [q01' q23' k01' k23']
            # v storage per head pair:
            #   [v_even(0:64) | ones(64:66) | gap(66:97) | v_odd(97:161)]
            # The AV lhsT is a 128-wide window: even head -> cols 0..127, so y
            # lands in psum rows 0..63 with the denominator in row 64; odd
            # head -> cols 33..160, so y lands in rows 64..127 with the
            # denominator (ones col 65) in row 32. Engine ops require
            # 32-aligned partition starts, so denominator rows must be 32/64.
            v_s = per.tile([128, NT, 2, 161], bf16)
            yT_s = per.tile([128, 2, T], bf16)

            # ---- startup DMAs in priority order: the first projection chain
            # needs w1 block cb + x block cb, so interleave them ----
            xts0 = []
            for cb in range(8):
                nc.sync.dma_start(out=w1_s[:, cb, :], in_=w1[cb * 128:(cb + 1) * 128, :])
                xt = xq.tile([128, TS], bf16, tag="xts")
                nc.sync.dma_start(out=xt, in_=xT[cb * 128:(cb + 1) * 128, 0:TS])
                xts0.append(xt)
            nc.sync.dma_start(out=b1_s, in_=b1[:])
            nc.sync.dma_start(out=pt2_s, in_=pt2[:])
            nc.sync.dma_start(out=cos_s, in_=cosb[:])
            nc.sync.dma_start(out=sin_s, in_=sinb[:])
            nc.sync.dma_start(out=msk_s, in_=msk[:])
            for cb in range(8):
                nc.sync.dma_start(out=wv_s[:, cb, :], in_=wv[cb * 128:(cb + 1) * 128, :])
            for jb in range(2):
                nc.sync.dma_start(out=wp_s[:, jb, :], in_=wp[jb * 128:(jb + 1) * 128, :])
            nc.sync.dma_start(out=idn_s, in_=idn[:])
            nc.gpsimd.memset(v_s.rearrange("p a b c -> p (a b c)"), 1.0)
            nc.gpsimd.memset(ones_s, 1.0)

            def v_win(tb, h):
                # 128-wide lhsT window into the [v_even |1|1| v_odd] slot
                return v_s[:, tb, h // 2, 33 * (h % 2):33 * (h % 2) + 128]

            def emit_oproj_chunk(tb):
                for cs in range(2):
                    py = psP.tile([128, 512], f32, tag="py")
                    for jb in range(2):
                        nc.tensor.matmul(
                            py,
                            yT_s[:, jb, tb * 128:(tb + 1) * 128],
                            wp_s[:, jb, cs * 512:(cs + 1) * 512],
                            start=(jb == 0),
                            stop=(jb == 1),
                        )
                    ot = yo.tile([128, 512], bf16, tag="ot")
                    nc.vector.tensor_copy(ot, py)
                    nc.sync.dma_start(
                        out=out[tb * 128:(tb + 1) * 128, cs * 512:(cs + 1) * 512],
                        in_=ot,
                    )

            mul_queue = []

            def emit_av(js, h, pend, tail_state=None):
                nkb = 4 * js + 4
                qsl = slice(js * TS, (js + 1) * TS)
                po = psO.tile([128, 512], f32, tag="av")
                for kb, (pt, w0) in enumerate(pend):
                    nc.tensor.matmul(
                        po[:, w0:] if w0 else po,
                        v_win(kb, h),
                        pt[:, w0:] if w0 else pt,
                        start=(kb == 0),
                        stop=(kb == nkb - 1),
                        skip_group_check=True,
                    )
                if tail_state is not None:
                    # tail-critical normalize runs with no DMA hops (the
                    # PE-transpose/reciprocal/broadcast chain is emitted in
                    # the tail block, interleaved with warm-up matmuls)
                    srow = 64 - 32 * (h % 2)
                    yu = yup.tile([128, TS], f32, tag="yu")
                    nc.vector.tensor_copy(yu, po)
                    tail_state.update(yu=yu, srow=srow, h=h, js=js)
                    return
                # Release the psum bank fast: one copy grabs both the
                # unnormalized y rows and the denominator row (32-aligned
                # span), then the reciprocal bounce runs from SBUF on the
                # otherwise-idle gpsimd DMA queue.  The normalize multiply is
                # deferred (mul_queue) so the Vector queue never blocks on the
                # bounce latency.
                par = h % 2
                srow = 64 - 32 * par        # denominator row (32-aligned)
                prow = slice(par * 64, par * 64 + 64)  # y rows
                slot = h * 4 + js
                yu = yup.tile([128, TS], f32, tag="yu")
                nc.vector.tensor_copy(yu, po)
                nc.sync.dma_start(out=scr[slot:slot + 1, :], in_=yu[srow:srow + 1, :])
                rv = rr2.tile([64, 8], f32, tag="rv")
                nc.sync.dma_start(
                    out=rv,
                    in_=scr[slot:slot + 1, :].rearrange("a (p f) -> (a p) f", p=64),
                )
                rvr = rr2.tile([64, 8], f32, tag="rvr")
                nc.vector.reciprocal(rvr, rv)
                nc.sync.dma_start(
                    out=scr2[slot:slot + 1, :].rearrange("a (p f) -> (a p) f", p=64),
                    in_=rvr,
                )
                rb = rrb.tile([128, TS], f32, tag="rb")
                sc = scr2[slot:slot + 1, :]
                nc.gpsimd.dma_start(
                    out=rb[prow, :],
                    in_=bass.AP(tensor=sc.tensor, offset=sc.offset,
                                ap=[[0, 64]] + list(sc.ap[1:])),
                )

                def do_mul(tsub=None):
                    if tsub is None:
                        nc.vector.tensor_mul(yT_s[prow, h // 2, qsl],
                                             yu[prow, :], rb[prow, :])
                    else:
                        q0 = js * TS + tsub * 128
                        nc.vector.tensor_mul(
                            yT_s[prow, h // 2, q0:q0 + 128],
                            yu[prow, tsub * 128:(tsub + 1) * 128],
                            rb[prow, tsub * 128:(tsub + 1) * 128],
                        )

                mul_queue.append(do_mul)

            def phase1(ts):
                tsl = slice(ts * TS, (ts + 1) * TS)
                if ts == 0:
                    xts = xts0
                else:
                    xts = []
                    for cb in range(8):
                        xt = xq.tile([128, TS], bf16, tag="xts")
                        nc.sync.dma_start(out=xt, in_=xT[cb * 128:(cb + 1) * 128, tsl])
                        xts.append(xt)

                qkr_t = {}

                def emit_chain(jb):
                    ps = psA.tile([128, 512], f32, tag="mm")
                    for cb in range(8):
                        nc.tensor.matmul(
                            ps,
                            w1_s[:, cb, jb * 128:(jb + 1) * 128],
                            xts[cb],
                            start=(cb == 0),
                            stop=(cb == 7),
                        )
                    qkr = qkrp.tile([128, TS], bf16, tag="qkr")
                    nc.scalar.activation(qkr, ps, Ident, bias=b1_s[:, jb:jb + 1], scale=1.0)
                    qkr_t[jb] = qkr

                def emit_rope(jb):
                    psr = psA.tile([128, 512], f32, tag="mm")
                    nc.tensor.matmul(psr, pt2_s, qkr_t[jb], start=True, stop=True)
                    t1 = tp1.tile([128, TS], bf16, tag="t1")
                    nc.vector.tensor_mul(t1, qkr_t[jb], cos_s[:, tsl])
                    t2 = tp2.tile([128, TS], bf16, tag="t2")
                    nc.vector.tensor_mul(t2, psr, sin_s[:, tsl])
                    nc.vector.tensor_add(qk_s[:, jb, tsl], t1, t2)

                emit_chain(0)
                emit_chain(1)
                emit_rope(0)
                emit_chain(2)
                emit_rope(1)
                emit_chain(3)
                emit_rope(2)
                for tb2 in range(4):
                    tb = ts * 4 + tb2
                    psv = psA.tile([128, 512], f32, tag="mm")
                    for cb in range(8):
                        nc.tensor.matmul(
                            psv[:, :256],
                            xts[cb][:, tb2 * 128:(tb2 + 1) * 128],
                            wv_s[:, cb, :],
                            start=(cb == 0),
                            stop=(cb == 7),
                        )
                    # one fused copy: psum [p, pr, par, d] -> slot cols
                    # {0:64, 97:161} of both pairs (par stride 97, pair 161)
                    s0 = v_s[:, tb, 0, 0:64]
                    dst = bass.AP(tensor=s0.tensor, offset=s0.offset,
                                  ap=[s0.ap[0], [161, 2], [97, 2], [1, 64]])
                    src = psv[:, :256].rearrange("p (pr par d) -> p pr par d",
                                                 par=2, d=D)
                    nc.vector.tensor_copy(dst, src)
                emit_rope(3)

            def attention(js, tail_state=None):
                nkb = 4 * js + 4
                pend_prev = None
                # yT of the previous super must be fully written before this
                # super's oproj chunks (emitted below) read it
                while mul_queue:
                    mul_queue.pop(0)()
                for h in range(G):
                    par = h % 2
                    hrow = slice(par * 64, par * 64 + 64)
                    qT = qk_s[hrow, h // 2, :]
                    kT = qk_s[hrow, 2 + h // 2, :]
                    pend = []
                    masks = []
                    for kb in range(nkb):
                        roff = kb - 4 * js
                        w0 = 128 * roff if roff > 0 else 0
                        pss = psA.tile([128, 512], f32, tag="mm")
                        nc.tensor.matmul(
                            pss[:, w0:] if w0 else pss,
                            kT[:, kb * 128:(kb + 1) * 128],
                            qT[:, js * TS + w0:(js + 1) * TS],
                            start=True,
                            stop=True,
                        )
                        pt = ptp.tile([128, TS], bf16, tag="pt")
                        nc.scalar.activation(pt[:, w0:], pss[:, w0:], Exp, scale=0.125)
                        if roff >= 0:
                            masks.append((pt, w0))
                        pend.append((pt, w0))
                    if h > 0:
                        emit_av(js, h - 1, pend_prev)
                    if len(mul_queue) > 1:
                        mul_queue.pop(0)()
                    for pt, w0 in masks:
                        nc.vector.tensor_mul(
                            pt[:, w0:w0 + 128], pt[:, w0:w0 + 128], msk_s
                        )
                    if js >= 1:
                        emit_oproj_chunk(4 * (js - 1) + h)
                    pend_prev = pend
                emit_av(js, G - 1, pend_prev, tail_state=tail_state)

            # attention(js) is emitted after phase1(js+1): the projection
            # matmuls give Scalar a head start on the attention exps, so the
            # AV chains never wait on the exp stream
            phase1(0)
            for ts in range(1, NTS):
                phase1(ts)
                attention(ts - 1)
            tstate = {}
            attention(NTS - 1, tail_state=tstate)
            srow, yu, hh = tstate["srow"], tstate["yu"], tstate["h"]
            prow = slice((hh % 2) * 64, (hh % 2) * 64 + 64)

            def warm(n):
                # keep the PE p-state hot across the normalize chain
                for w in range(n):
                    pw = psA.tile([128, 512], f32, tag="mm")
                    nc.tensor.matmul(pw, pt2_s, qk_s[:, 0, 0:TS], start=True, stop=True)

            warm(3)
            # spread the denominator onto partitions: PE-transpose each
            # 128-wide block of the S row into a [128, 4] psum tile
            tp = psA.tile([128, 512], f32, tag="mm")
            for qb in range(4):
                nc.tensor.matmul(
                    tp[:, qb:qb + 1],
                    yu[srow:srow + 1, qb * 128:(qb + 1) * 128],
                    idn_s[srow:srow + 1, srow:srow + 1],
                    is_transpose=True, start=True, stop=True,
                    skip_group_check=True,
                )
            rtp = rr2.tile([128, 4], f32, tag="rtp")
            nc.vector.reciprocal(rtp, tp[:, 0:4])
            for m in mul_queue:
                m()
            mul_queue.clear()
            warm(8)
            # transpose 1/S back to a single row, then broadcast it across
            # 128 partitions with a K=1 ones matmul
            tp2 = psA.tile([128, 512], f32, tag="mm")
            for qb in range(4):
                nc.tensor.matmul(
                    tp2[0:1, qb * 128:(qb + 1) * 128],
                    rtp[:, qb:qb + 1],
                    idn_s,
                    is_transpose=True, start=True, stop=True,
                    skip_group_check=True,
                )
            rcps = rr2.tile([64, TS], f32, tag="rcp")
            nc.vector.tensor_copy(rcps[0:1, :], tp2[0:1, :])
            rbps = psO.tile([128, 512], f32, tag="av")
            nc.tensor.matmul(rbps, ones_s[0:1, :], rcps[0:1, :],
                             start=True, stop=True)
            for i, tb in enumerate(range(4 * (NTS - 1), NT)):
                q0 = tstate["js"] * TS + i * 128
                nc.vector.tensor_mul(
                    yT_s[prow, hh // 2, q0:q0 + 128],
                    yu[prow, i * 128:(i + 1) * 128],
                    rbps[prow, i * 128:(i + 1) * 128],
                )
                emit_oproj_chunk(tb)

    _split_multiwaits(nc)
    return nc


def _host_inputs(x, W_attn, b_attn, W_proj):
    f32 = np.float32
    import ml_dtypes

    bf16 = ml_dtypes.bfloat16

    inv = (1.0 / (10000.0 ** (np.arange(0, D, 2, dtype=f32) / f32(D)))).astype(f32)
    t = np.arange(T, dtype=f32)
    ang = np.outer(inv, t).astype(f32)            # [32, T]
    cos32, sin32 = np.cos(ang).astype(f32), np.sin(ang).astype(f32)
    cosb = np.tile(cos32, (4, 1)).astype(bf16)    # [128, T], row p -> freq p%32
    sinb = np.tile(sin32, (4, 1)).astype(bf16)

    kk = np.arange(128)[:, None]
    qq = np.arange(128)[None, :]
    msk128 = np.where(qq >= kk, f32(1), f32(0)).astype(bf16)  # [128,128]

    p64 = np.zeros((D, D), dtype=f32)
    for d in range(32):
        p64[d, d + 32] = -1.0
        p64[d + 32, d] = 1.0
    pt2 = np.zeros((128, 128), dtype=f32)
    pt2[:64, :64] = p64.T
    pt2[64:, 64:] = p64.T
    pt2 = pt2.astype(bf16)

    xTs = [np.ascontiguousarray(x[b].T).astype(bf16) for b in range(B)]

    per_g = []
    for g in range(G):
        hs = [4 * g + j for j in range(G)]
        qcols = [W_attn[:, h * D:(h + 1) * D] for h in hs]
        kcols = [W_attn[:, C + h * D:C + (h + 1) * D] for h in hs]
        qb = [b_attn[h * D:(h + 1) * D] for h in hs]
        kb_ = [b_attn[C + h * D:C + (h + 1) * D] for h in hs]
        w1 = np.concatenate(
            [qcols[0], qcols[1], qcols[2], qcols[3], kcols[0], kcols[1], kcols[2], kcols[3]],
            axis=1,
        ).astype(bf16)                             # [C, 512]: [q01 q23 k01 k23]
        b1 = np.concatenate(qb + kb_).astype(f32).reshape(4, 128).T.copy()  # [128, 4]
        wv_ = W_attn[:, 2 * C + 256 * g:2 * C + 256 * (g + 1)].astype(bf16)
        wp_ = W_proj[256 * g:256 * (g + 1), :].astype(bf16)
        per_g.append((w1, b1, wv_, wp_))

    shared = dict(cosb=cosb, sinb=sinb, msk=msk128, pt2=pt2,
                  idn=np.eye(128, dtype=f32))
    in_maps = []
    for i in range(NCORES):
        b, g = i // 4, i % 4
        w1, b1, wv_, wp_ = per_g[g]
        in_maps.append(dict(xT=xTs[b], w1=w1, b1=b1, wv=wv_, wp=wp_, **shared))
    return in_maps


def kernel(x, W_attn, b_attn, W_proj, b_proj):
    from concourse.bass_utils import run_bass_kernel_spmd

    x = np.asarray(x, dtype=np.float32)
    W_attn = np.asarray(W_attn, dtype=np.float32)
    b_attn = np.asarray(b_attn, dtype=np.float32)
    W_proj = np.asarray(W_proj, dtype=np.float32)
    b_proj = np.asarray(b_proj, dtype=np.float32)

    if "nc" not in _cached:
        _cached["nc"] = _build()
    nc = _cached["nc"]

    in_maps = _host_inputs(x, W_attn, b_attn, W_proj)
    res = run_bass_kernel_spmd(nc, in_maps, core_ids=list(range(NCORES)))
    _cached["last_results"] = res

    const = (b_proj + b_attn[2 * C:] @ W_proj).astype(np.float32)
    y = np.empty((B, T, C), dtype=np.float32)
    for b in range(B):
        acc = res.results[4 * b]["out"].astype(np.float32)
        for g in range(1, 4):
            acc = acc + res.results[4 * b + g]["out"].astype(np.float32)
        y[b] = acc + const
    return y
